# revision 26
# baseline (speedup 1.0000x reference)
"""Masked multi-head attention (B=4, T=2048, D=1024, H=16) on 8 trn2 NeuronCores.

Sharding: core c handles batch b = c//2 and head-group g = c%2 (8 heads, 512
of the 1024 model dims).  Each core runs the fused QKV projection for its
head-group over its batch, causal+padding-masked attention for its 8 heads,
and a partial out-projection (its 512 rows of W_o).  Device emits two partial
y tensors (head-pair halves); the host sums the four partials per batch.

Device algorithm (per core), all matmuls bf16 with f32 PSUM accumulation:
  - qT,kT  = (x @ Wq|k)^T computed directly in [dims, tok] layout
             (lhsT = W chunk, rhs = xT chunk), bias added per-partition.
  - V      computed in natural [tok, dims] layout, packed into
             V_aug = [V | 1] (even heads) or [1 | V] (odd heads) so A@V_aug
             also yields the softmax row-sums replicated across 64 partitions.
  - scores S^T[k, q] per 128-key block kb: the two heads of a pair use PE
             row groups 0:64 / 64:128 and separate PSUM banks, so the pair
             of score matmuls runs CONCURRENTLY in the array when adjacent
             in the PE stream (row-tiled).  Keys >= 1792 (padded) never
             computed; causal handled by skipping blocks + an additive
             -1e30 mask on the 128 diagonal columns (DVE add on PSUM before
             exp, keeping the exp->A@V path short).
  - ctx^T  accumulated over key blocks in PSUM; A@V pipelined one key-block
             behind scores so score pairs stay adjacent (concurrent).
  - y      = ctx @ W_o rows in two head-pair halves (partA: c4 0,1 with
             bias; partB: c4 2,3) -> separate DRAM outputs ya/yb summed on
             host.  partA runs as PE filler during pair-2 attention, partB
             during pair-3; only the last 512 queries' partB remains as tail.

Scheduling: Tile's priority scheduler picks the lowest-priority READY PE
instruction; emission order sets priority.  Emitting scores(kb) before
A@V(kb-1) keeps score pairs adjacent; qk/v/proj tiles are woven as fillers
so the PE never idles while ScalarE exps (1.1us each) run.  A scratch-tile
warmup burst keeps PE busy from ~8us (HAM warm) while inputs DMA in
1024-column chunks ordered by first use.
"""

import os
import sys

sys.path.insert(0, "/opt/trn_rl_repo")

from contextlib import ExitStack

import ml_dtypes
import numpy as np

import concourse.bass as bass
import concourse.tile as tile
from concourse import bacc, mybir
from concourse.bass_utils import run_bass_kernel_spmd

B, T, D, H, HD = 4, 2048, 1024, 16, 64
N_CORES = 8
NH = H // 2            # heads per core = 8
GD = NH * HD           # head-group width = 512
TK = 14                # valid 128-key blocks (keys < 1792; rest padded)
NPAD = 256             # padded key positions at the end
BF16 = mybir.dt.bfloat16
F32 = mybir.dt.float32
AF = mybir.ActivationFunctionType

_CACHE = {}


def _build():
    nc = bacc.Bacc("TRN2", target_bir_lowering=False, debug=False,
                   num_devices=N_CORES)
    # xT packed as [128, (nt, d) blocks of 512]; wq packed as
    # [128, m0|m4|V|m1|m5|m2|m6|m3|m7 blocks] -- both host-reordered so every
    # DMA chunk is fully contiguous (large descriptors, ordered by first use).
    xT_d = nc.dram_tensor("xT", [128, 8 * T], BF16, kind="ExternalInput").ap()
    wqkv_d = nc.dram_tensor("wqkv", [128, 12 * 1024], BF16,
                            kind="ExternalInput").ap()
    wo_d = nc.dram_tensor("wo", [GD, D], BF16, kind="ExternalInput").ap()
    bqk_d = nc.dram_tensor("bqk", [128, 8], F32, kind="ExternalInput").ap()
    bv_d = nc.dram_tensor("bv", [GD], F32, kind="ExternalInput").ap()
    bo_d = nc.dram_tensor("bo", [D], F32, kind="ExternalInput").ap()
    ya_d = nc.dram_tensor("ya", [T, D], F32, kind="ExternalOutput").ap()
    # pairs-2,3 partial for the last 512 queries; bf16 partials halve the
    # tail DMA (host adds in f32)
    yb_d = nc.dram_tensor("yb", [T, D], BF16, kind="ExternalOutput").ap()

    def bcast128(src_ap):
        """DMA access pattern replicating a 1-D dram vector over 128 partitions."""
        return bass.AP(tensor=src_ap.tensor, offset=src_ap.offset,
                       ap=[[0, 128]] + list(src_ap.ap))

    with tile.TileContext(nc) as tc, ExitStack() as ctx:
        pers = ctx.enter_context(tc.tile_pool(name="pers", bufs=1))
        ps_pool = ctx.enter_context(tc.tile_pool(name="ps", bufs=2, space="PSUM"))
        esp = ctx.enter_context(tc.tile_pool(name="es", bufs=4))
        nrmp = ctx.enter_context(tc.tile_pool(name="nrm", bufs=2))
        yp = ctx.enter_context(tc.tile_pool(name="yp", bufs=4))

        # ---- persistent tiles ----
        wo_sb = pers.tile([128, 4, D], BF16)          # W_o rows, 4 chunks of 128
        bqk_sb = pers.tile([128, 8], F32)             # q|k bias per col-tile
        bv_bc = pers.tile([128, GD], F32)             # v bias bcast over tokens
        bo_bc = pers.tile([128, D], F32)              # out bias bcast over tokens
        bandneg = pers.tile([128, 2, 128], F32)       # 0 where col>=row else -1e30, x2
        qT_sb = pers.tile([128, 4, T], BF16)          # qT per head pair
        kT_sb = pers.tile([128, 4, T], BF16)          # kT per head pair (own tile:
                                                      # scores read lhsT from kT and
                                                      # rhs from qT concurrently)
        vaug = pers.tile([128, 2, 4, TK, 128], BF16)  # V_aug[par, hp, key chunk]
        xT_sb = pers.tile([128, 8 * T], BF16)         # packed (nt, d) blocks
        wq_sb = pers.tile([128, 12 * 1024], BF16)     # packed m/V blocks
        scr = pers.tile([128, 512], BF16)             # PE warmup scratch

        QKOFF = {0: 0, 4: 1024, 1: 6144, 5: 7168, 2: 8192, 6: 9216,
                 3: 10240, 7: 11264}
        VOFF = 2048

        def wq_qk(m, d8):
            return wq_sb[:, QKOFF[m] + 128 * d8:QKOFF[m] + 128 * (d8 + 1)]

        def xT_nt(nt, d8):
            return xT_sb[:, (nt * 8 + d8) * 512:(nt * 8 + d8) * 512 + 512]
        ctxn = pers.tile([128, 4, 4, 512], BF16)      # normalized ctx^T chunks

        # ---- PE warmup: 12 matmuls on a zero scratch tile, starting as soon
        #      as the DVE memset lands (~8us), so the HAM clock is at 8/8 and
        #      the pipeline hot when the first real matmul's data arrives ----
        nc.vector.memset(scr[:], 0.0)
        for g in range(5):
            wps = ps_pool.tile([128, 512], F32, tag="sc", name=f"warm_{g}")
            for i in range(4):
                nc.tensor.matmul(wps[:], lhsT=scr[:, 0:128], rhs=scr[:],
                                 start=(i == 0), stop=(i == 3))

        # ---- loads: contiguous 1024-col chunks ordered by first use ----
        def chunks(sb, dram, lo, hi, step=1024):
            for a in range(lo, hi, step):
                nc.sync.dma_start(out=sb[:, a:a + step], in_=dram[:, a:a + step])

        chunks(wq_sb, wqkv_d, 0, 1024)          # m0
        chunks(xT_sb, xT_d, 0, 2048)            # nt0 d0..3
        chunks(wq_sb, wqkv_d, 1024, 2048)       # m4
        chunks(xT_sb, xT_d, 2048, 4096)         # nt0 d4..7
        nc.sync.dma_start(out=bqk_sb[:], in_=bqk_d)
        nc.sync.dma_start(out=bv_bc[:], in_=bcast128(bv_d))
        chunks(wq_sb, wqkv_d, 2048, 6144)       # V columns
        chunks(wq_sb, wqkv_d, 6144, 8192)       # m1 + m5 (pair-1 fillers read nt0)
        chunks(xT_sb, xT_d, 4096, 8192)         # nt1
        chunks(xT_sb, xT_d, 8192, 12288)        # nt2
        chunks(wq_sb, wqkv_d, 8192, 10240)      # m2 + m6
        chunks(xT_sb, xT_d, 12288, 16384)       # nt3
        chunks(wq_sb, wqkv_d, 10240, 12288)     # m3 + m7
        for c4 in range(4):
            nc.sync.dma_start(out=wo_sb[:, c4, :], in_=wo_d[128 * c4:128 * (c4 + 1), :])
        nc.sync.dma_start(out=bo_bc[:], in_=bcast128(bo_d))
        # bandneg[k, :, j] = 0 where j >= k else -1e30 (additive causal mask
        # for the 128 diagonal columns, applied on PSUM before exp; two copies
        # so both heads' regions mask with a single strided DVE add)
        nc.gpsimd.memset(bandneg[:], 0.0)
        for a in range(2):
            nc.gpsimd.affine_select(out=bandneg[:, a, :], in_=bandneg[:, a, :],
                                    compare_op=mybir.AluOpType.is_ge, fill=-1e30,
                                    base=0, pattern=[[1, 128]], channel_multiplier=-1)
        nc.gpsimd.memset(vaug[:, 0, :, :, 64:128], 1.0)   # even heads: [V | 1]
        nc.gpsimd.memset(vaug[:, 1, :, :, 0:64], 1.0)     # odd heads:  [1 | V]

        # ---- QKV projection pieces, emitted as PE fillers ----
        def qk_tile(m, nt):
            # k columns (m >= 4) beyond token 1792 are fully padded: never read
            w = 256 if (m >= 4 and nt == 3) else 512
            ps = ps_pool.tile([128, 512], F32, tag="p1", name=f"p1_{m}_{nt}")
            for d8 in range(8):
                nc.tensor.matmul(ps[:, 0:w], lhsT=wq_qk(m, d8),
                                 rhs=xT_nt(nt, d8)[:, 0:w],
                                 start=(d8 == 0), stop=(d8 == 7))
            dst = qT_sb if m < 4 else kT_sb
            nc.vector.tensor_scalar_add(dst[:, m % 4, 512 * nt:512 * nt + w],
                                        ps[:, 0:w], bqk_sb[:, m:m + 1])

        def v_tile(t16):
            ps = ps_pool.tile([128, 512], F32, tag="p1", name=f"p1v_{t16}")
            nt, to = t16 // 4, 128 * (t16 % 4)
            for d8 in range(8):
                nc.tensor.matmul(ps[:],
                                 lhsT=xT_sb[:, (nt * 8 + d8) * 512 + to:(nt * 8 + d8) * 512 + to + 128],
                                 rhs=wq_sb[:, VOFF + 512 * d8:VOFF + 512 * (d8 + 1)],
                                 start=(d8 == 0), stop=(d8 == 7))
            psv = ps.rearrange("p (hp par d) -> p hp par d", par=2, d=64)
            bvv = bv_bc.rearrange("p (hp par d) -> p hp par d", par=2, d=64)
            nc.vector.tensor_add(vaug[:, 0, :, t16, 0:64], psv[:, :, 0, :],
                                 bvv[:, :, 0, :])
            nc.vector.tensor_add(vaug[:, 1, :, t16, 64:128], psv[:, :, 1, :],
                                 bvv[:, :, 1, :])

        def av_pair(c, qt, kmax, cps, kb, est):
            off = max(0, 128 * kb - 512 * qt)
            w = 512 - off
            for par in (0, 1):
                nc.tensor.matmul(cps[par][:, off:512],
                                 lhsT=vaug[:, par, c, kb, :],
                                 rhs=est[:, 512 * par:512 * par + w],
                                 start=(kb == 0), stop=(kb == kmax))

        def attention_qt(c, qt, fillers=()):
            """Scores + exp + A@V for q-tile qt of head pair c.  The score
            pair is emitted back-to-back (concurrent row-tiled matmuls);
            A@V runs one key block behind so nothing splits the pair.
            `fillers` are independent PE work woven between key blocks to
            absorb the exp latency."""
            kmax = min(4 * qt + 3, TK - 1)
            fillers = list(fillers)
            nkb = kmax + 1
            fill_every = max(1, nkb // (len(fillers) + 1)) if fillers else 0
            cps = [ps_pool.tile([128, 512], F32, tag="cps", name=f"cps_{c}_{qt}_{i}")
                   for i in range(2)]
            prev = None
            for kb in range(nkb):
                if c == 0 and qt == kb // 4:   # JIT V chunks during pair 0
                    v_tile(kb)
                # diagonal blocks only need columns q >= 128*kb of the q-tile
                off = max(0, 128 * kb - 512 * qt)
                w = 512 - off
                psc = ps_pool.tile([128, 1024], F32, tag="sc", name=f"sc_{c}_{qt}_{kb}")
                for par in (0, 1):
                    r = 64 * par
                    nc.tensor.matmul(
                        psc[:, 512 * par:512 * par + w],
                        lhsT=kT_sb[r:r + 64, c, 128 * kb:128 * (kb + 1)],
                        rhs=qT_sb[r:r + 64, c, 512 * qt + off:512 * (qt + 1)],
                        start=True, stop=True)
                if kb >= 4 * qt:  # additive mask on the diagonal 128 columns
                    # one strided DVE add covers both heads' regions so the
                    # exp never waits on a second queued DVE op
                    pv = psc.rearrange("p (a b) -> p a b", a=2, b=512)
                    nc.vector.tensor_add(pv[:, :, 0:128], pv[:, :, 0:128],
                                         bandneg[:])
                est = esp.tile([128, 1024], BF16, tag="es", name=f"es_{c}_{qt}_{kb}")
                if w < 512:
                    # diagonal blocks: exp only the two live regions (one
                    # 2-region strided activation, skipping [w:512] garbage)
                    ev = est.rearrange("p (a b) -> p a b", a=2, b=512)
                    sv = psc.rearrange("p (a b) -> p a b", a=2, b=512)
                    nc.scalar.activation(ev[:, :, 0:w], sv[:, :, 0:w], AF.Exp,
                                         scale=float(1.0 / np.sqrt(HD)))
                else:
                    nc.scalar.activation(est[:], psc[:], AF.Exp,
                                         scale=float(1.0 / np.sqrt(HD)))
                if prev is not None:
                    av_pair(c, qt, kmax, cps, *prev)
                prev = (kb, est)
                if fillers and fill_every and kb % fill_every == fill_every - 1:
                    fillers.pop(0)()
            av_pair(c, qt, kmax, cps, *prev)
            for f in fillers:
                f()
            return cps

        def normalize(c, qt, cps):
            stg = nrmp.tile([128, 1024], F32, tag="stg", name=f"stg_{c}_{qt}")
            nc.vector.tensor_copy(stg[:, 0:512], cps[0][:])
            nc.vector.tensor_copy(stg[:, 512:1024], cps[1][:])
            # even head: ctx rows 0:64, sums rows 64:128 (V_aug = [V|1])
            # odd head:  sums rows 0:64, ctx rows 64:128 (V_aug = [1|V])
            # -> swap sums across partitions + gather ctx into one tile so a
            #    single full-width multiply normalizes both heads.  The final
            #    round's DMAs ride the Activation HWDGE ring, which is idle at
            #    the kernel tail (the sync ring is jammed with y stores).
            last = (c, qt) == (3, 3)
            eng = nc.scalar if last else nc.sync
            sums = nrmp.tile([128, 512], F32, tag="sums", name=f"sums_{c}_{qt}")
            eng.dma_start(out=sums[0:64, :], in_=stg[64:128, 0:512])
            eng.dma_start(out=sums[64:128, :], in_=stg[0:64, 512:1024])
            if last:
                # pipeline the tail: recip/multiply in 128-col chunks straight
                # from stg (no ctxc DMA on the critical chain) so the first
                # final-projection matmul starts ~2us earlier
                for j in range(2):
                    nc.vector.reciprocal_approx_fast(sums[:, 256 * j:256 * (j + 1)],
                                                     sums[:, 256 * j:256 * (j + 1)])
                    for i in range(2):
                        o = 256 * j + 128 * i
                        nc.vector.tensor_mul(ctxn[0:64, c, qt, o:o + 128],
                                             stg[0:64, o:o + 128],
                                             sums[0:64, o:o + 128])
                        nc.vector.tensor_mul(ctxn[64:128, c, qt, o:o + 128],
                                             stg[64:128, 512 + o:512 + o + 128],
                                             sums[64:128, o:o + 128])
            else:
                ctxc = nrmp.tile([128, 512], F32, tag="ctxc", name=f"ctxc_{c}_{qt}")
                eng.dma_start(out=ctxc[0:64, :], in_=stg[0:64, 0:512])
                eng.dma_start(out=ctxc[64:128, :], in_=stg[64:128, 512:1024])
                nc.vector.reciprocal_approx_fast(sums[:], sums[:])   # in place
                nc.vector.tensor_mul(ctxn[:, c, qt, :], ctxc[:], sums[:])

        def proj_part(t16, no, half):
            """Out-projection group for one [128 tok, 512] y half-tile.
            half None = all 4 head-pairs (+bias) -> ya.  For the last q-tile
            the group is split so most of it runs before the final attention
            round: half 0 = head-pairs 0,1 (+bias) -> ya; half 1 = pairs
            2,3 -> yb; the host sums the partials."""
            def emit():
                qt, o = t16 // 4, 128 * (t16 % 4)
                # tail groups cycle through all psum tags so their first
                # (pair-2) matmuls prefill while the final normalize chain
                # runs; attention tiles are done by then
                tag = ("p1", "sc", "cps")[(t16 * 2 + no) % 3] \
                    if (half == 1 and t16 >= 12) else "p1"
                ps = ps_pool.tile([128, 512], F32, tag=tag,
                                  name=f"yps_{t16}_{no}_{half}")
                cc = {None: (0, 1, 2, 3), 0: (0, 1), 1: (2, 3)}[half]
                for i, c4 in enumerate(cc):
                    nc.tensor.matmul(ps[:], lhsT=ctxn[:, c4, qt, o:o + 128],
                                     rhs=wo_sb[:, c4, 512 * no:512 * (no + 1)],
                                     start=(i == 0), stop=(i == len(cc) - 1))
                to_yb = half == 1 or (half is None and qt == 2)
                if to_yb:
                    yt = yp.tile([128, 512], BF16, tag="yh", name=f"yh_{t16}_{no}")
                else:
                    yt = yp.tile([128, 512], F32, tag="y", name=f"y_{t16}_{no}_{half}")
                if half == 1:
                    nc.vector.tensor_copy(yt[:], ps[:])
                else:
                    nc.vector.tensor_add(yt[:], ps[:], bo_bc[:, 512 * no:512 * (no + 1)])
                yd = yb_d if to_yb else ya_d
                nc.sync.dma_start(out=yd[128 * t16:128 * (t16 + 1), 512 * no:512 * (no + 1)],
                                  in_=yt[:])
            return emit

        # ---- interleaved schedule ----
        for c in range(4):
            for qt in range(4):
                if c == 0:
                    qk_tile(0, qt)
                    qk_tile(4, qt)
                    fillers = [lambda n=qt: qk_tile(1, n),
                               lambda n=qt: qk_tile(5, n)]
                elif c == 1:
                    fillers = [lambda n=qt: qk_tile(2, n),
                               lambda n=qt: qk_tile(6, n)]
                elif c == 2:
                    # pair-3 qk (prereqs for pair-3 rounds) + the last
                    # q-tile's first half-proj (ctxn(0..1, 3) is complete
                    # once pair 1 finished)
                    fillers = [lambda n=qt: qk_tile(3, n),
                               lambda n=qt: qk_tile(7, n)]
                    fillers += [proj_part(11 + qt, no, 0) for no in range(2)] \
                        if qt > 0 else []
                else:
                    if qt == 0:
                        fillers = [proj_part(15, no, 0) for no in range(2)]
                    elif qt == 1:
                        fillers = [proj_part(t16, no, None)
                                   for t16 in range(0, 4) for no in range(2)]
                    else:
                        fillers = [proj_part(t16, no, None)
                                   for t16 in range(4 * (qt - 1), 4 * qt)
                                   for no in range(2)]
                cps = attention_qt(c, qt, fillers)
                normalize(c, qt, cps)
        for t16 in range(12, 16):   # tail: last queries' second half-proj
            for no in range(2):
                proj_part(t16, no, 1)()

    nc.compile()
    return nc


def _reference_np(x, W_qkv, b_qkv, W_o, b_o, key_padding_mask):
    """Numpy fallback for inputs that do not match the compiled assumptions."""
    b_, t_, d_ = x.shape
    hd = d_ // H
    qkv = x.astype(np.float64) @ W_qkv.astype(np.float64) + b_qkv
    q, k, v = np.split(qkv, 3, axis=-1)

    def heads(t):
        return t.reshape(b_, t_, H, hd).transpose(0, 2, 1, 3)

    q, k, v = heads(q), heads(k), heads(v)
    s = np.einsum("bhqd,bhkd->bhqk", q, k) / np.sqrt(hd)
    causal = np.triu(np.ones((t_, t_), bool), k=1)
    mask = key_padding_mask[:, None, None, :] | causal[None, None]
    s = np.where(mask, -np.inf, s)
    s = s - s.max(axis=-1, keepdims=True)
    e = np.exp(s)
    with np.errstate(invalid="ignore"):
        a = e / e.sum(axis=-1, keepdims=True)
    ctx = np.einsum("bhqk,bhkd->bhqd", a, v)
    y = ctx.transpose(0, 2, 1, 3).reshape(b_, t_, d_) @ W_o.astype(np.float64) + b_o
    return y.astype(np.float32)


def kernel(x, W_qkv, b_qkv, W_o, b_o, key_padding_mask):
    x = np.asarray(x)
    W_qkv, b_qkv = np.asarray(W_qkv), np.asarray(b_qkv)
    W_o, b_o = np.asarray(W_o), np.asarray(b_o)
    key_padding_mask = np.asarray(key_padding_mask)

    expected_mask = np.zeros((B, T), bool)
    expected_mask[:, T - NPAD:] = True
    if (x.shape != (B, T, D) or not np.array_equal(key_padding_mask, expected_mask)):
        return _reference_np(x, W_qkv, b_qkv, W_o, b_o, key_padding_mask)

    if "nc" not in _CACHE:
        _CACHE["nc"] = _build()
    nc = _CACHE["nc"]

    bf = ml_dtypes.bfloat16
    in_maps = []
    for c in range(N_CORES):
        b, g = divmod(c, 2)
        cols = slice(g * GD, (g + 1) * GD)
        wq = np.concatenate([W_qkv[:, cols], W_qkv[:, D + g * GD:D + (g + 1) * GD],
                             W_qkv[:, 2 * D + g * GD:2 * D + (g + 1) * GD]],
                            axis=1).astype(bf)
        bq = np.concatenate([b_qkv[cols], b_qkv[D + g * GD:D + (g + 1) * GD]])
        xT = np.ascontiguousarray(x[b].T).astype(bf)
        # pack wq columns: m0 | m4 | V | m1 m5 m2 m6 m3 m7 (d-major inside)
        wq_blocks = []
        for m in (0, 4):
            wq_blocks += [wq[128 * d:128 * (d + 1), 128 * m:128 * (m + 1)]
                          for d in range(8)]
        wq_blocks += [wq[128 * d:128 * (d + 1), 1024:1536] for d in range(8)]
        for m in (1, 5, 2, 6, 3, 7):
            wq_blocks += [wq[128 * d:128 * (d + 1), 128 * m:128 * (m + 1)]
                          for d in range(8)]
        wq_p = np.concatenate(wq_blocks, axis=1)
        # pack xT columns: (nt, d) blocks of 512 tokens
        xT_p = np.concatenate([xT[128 * d:128 * (d + 1), 512 * nt:512 * (nt + 1)]
                               for nt in range(4) for d in range(8)], axis=1)
        in_maps.append({
            "xT": np.ascontiguousarray(xT_p),
            "wqkv": np.ascontiguousarray(wq_p),
            "wo": np.ascontiguousarray(W_o[g * GD:(g + 1) * GD, :]).astype(bf),
            "bqk": np.ascontiguousarray(bq.reshape(8, 128).T.astype(np.float32)),
            "bv": np.ascontiguousarray(b_qkv[2 * D + g * GD:2 * D + (g + 1) * GD]).astype(np.float32),
            "bo": np.ascontiguousarray(0.5 * b_o).astype(np.float32),
        })

    trace = bool(os.environ.get("MHA_TRACE"))
    if trace:
        _register_ntff_hook()
    res = run_bass_kernel_spmd(nc, in_maps, core_ids=list(range(N_CORES)),
                               trace=trace)
    if trace:
        _CACHE["exec_time_ns"] = res.exec_time_ns

    y = np.empty((B, T, D), np.float32)
    for b in range(B):
        ya0, ya1 = res.results[2 * b]["ya"], res.results[2 * b + 1]["ya"]
        yb0 = res.results[2 * b]["yb"].astype(np.float32)
        yb1 = res.results[2 * b + 1]["yb"].astype(np.float32)
        y[b] = ya0 + ya1
        y[b, 1024:1536] = yb0[1024:1536] + yb1[1024:1536]  # qt2 rows ride yb
        y[b, 1536:] += yb0[1536:] + yb1[1536:]             # pairs-2,3 partials
    return y


def _register_ntff_hook():
    """antenv.axon_hooks is absent in this container; synthesize it so
    run_bass_kernel_spmd(trace=True) can NTFF-profile via ctypes."""
    import types

    if "antenv.axon_hooks" in sys.modules:
        return
    sys.path.insert(0, "/root/.axon_site")
    from trn_agent_boot.trn_boot import _ntff_profile_via_ctypes

    hook = _ntff_profile_via_ctypes("/opt/axon/libaxon_pjrt.so")
    mod = types.ModuleType("antenv.axon_hooks")
    mod._hook = hook
    mod.get_axon_ntff_profile_hook = lambda: mod._hook
    mod.set_axon_ntff_profile_hook = lambda h: setattr(mod, "_hook", h)
    sys.modules["antenv.axon_hooks"] = mod


# revision 27
# speedup vs baseline: 1.1819x; 1.1819x over previous
"""Masked multi-head attention (B=4, T=2048, D=1024, H=16) on 8 trn2 NeuronCores.

Sharding: core c handles batch b = c//2 and head-group g = c%2 (8 heads, 512
of the 1024 model dims).  Each core runs the fused QKV projection for its
head-group over its batch, causal+padding-masked attention for its 8 heads,
and a partial out-projection (its 512 rows of W_o).  Device emits two partial
y tensors (head-pair halves); the host sums the four partials per batch.

Device algorithm (per core), all matmuls bf16 with f32 PSUM accumulation:
  - qT,kT  = (x @ Wq|k)^T computed directly in [dims, tok] layout
             (lhsT = W chunk, rhs = xT chunk), bias added per-partition.
  - V      computed in natural [tok, dims] layout, packed into
             V_aug = [V | 1] (even heads) or [1 | V] (odd heads) so A@V_aug
             also yields the softmax row-sums replicated across 64 partitions.
  - scores S^T[k, q] per 128-key block kb: the two heads of a pair use PE
             row groups 0:64 / 64:128 and separate PSUM banks, so the pair
             of score matmuls runs CONCURRENTLY in the array when adjacent
             in the PE stream (row-tiled).  Keys >= 1792 (padded) never
             computed; causal handled by skipping blocks + an additive
             -1e30 mask on the 128 diagonal columns (DVE add on PSUM before
             exp, keeping the exp->A@V path short).
  - ctx^T  accumulated over key blocks in PSUM; A@V pipelined one key-block
             behind scores so score pairs stay adjacent (concurrent).
  - y      = ctx @ W_o rows in two head-pair halves (partA: c4 0,1 with
             bias; partB: c4 2,3) -> separate DRAM outputs ya/yb summed on
             host.  partA runs as PE filler during pair-2 attention, partB
             during pair-3; only the last 512 queries' partB remains as tail.

Scheduling: Tile's priority scheduler picks the lowest-priority READY PE
instruction; emission order sets priority.  Emitting scores(kb) before
A@V(kb-1) keeps score pairs adjacent; qk/v/proj tiles are woven as fillers
so the PE never idles while ScalarE exps (1.1us each) run.  A scratch-tile
warmup burst keeps PE busy from ~8us (HAM warm) while inputs DMA in
1024-column chunks ordered by first use.
"""

import os
import sys

sys.path.insert(0, "/opt/trn_rl_repo")

from contextlib import ExitStack

import ml_dtypes
import numpy as np

import concourse.bass as bass
import concourse.tile as tile
from concourse import bacc, mybir
from concourse.bass_utils import run_bass_kernel_spmd

B, T, D, H, HD = 4, 2048, 1024, 16, 64
N_CORES = 8
NH = H // 2            # heads per core = 8
GD = NH * HD           # head-group width = 512
TK = 14                # valid 128-key blocks (keys < 1792; rest padded)
NPAD = 256             # padded key positions at the end
BF16 = mybir.dt.bfloat16
F32 = mybir.dt.float32
AF = mybir.ActivationFunctionType

_CACHE = {}


def _build():
    nc = bacc.Bacc("TRN2", target_bir_lowering=False, debug=False,
                   num_devices=N_CORES)
    # xT packed as [128, (nt, d) blocks of 512]; wq packed as
    # [128, m0|m4|V|m1|m5|m2|m6|m3|m7 blocks] -- both host-reordered so every
    # DMA chunk is fully contiguous (large descriptors, ordered by first use).
    xT_d = nc.dram_tensor("xT", [128, 8 * T], BF16, kind="ExternalInput").ap()
    wqkv_d = nc.dram_tensor("wqkv", [128, 12 * 1024], BF16,
                            kind="ExternalInput").ap()
    wo_d = nc.dram_tensor("wo", [GD, D], BF16, kind="ExternalInput").ap()
    bqk_d = nc.dram_tensor("bqk", [128, 8], F32, kind="ExternalInput").ap()
    bv_d = nc.dram_tensor("bv", [GD], F32, kind="ExternalInput").ap()
    bo_d = nc.dram_tensor("bo", [D], F32, kind="ExternalInput").ap()
    ya_d = nc.dram_tensor("ya", [T, D], F32, kind="ExternalOutput").ap()
    # pairs-2,3 partial for the last 512 queries; bf16 partials halve the
    # tail DMA (host adds in f32)
    yb_d = nc.dram_tensor("yb", [T, D], BF16, kind="ExternalOutput").ap()

    def bcast128(src_ap):
        """DMA access pattern replicating a 1-D dram vector over 128 partitions."""
        return bass.AP(tensor=src_ap.tensor, offset=src_ap.offset,
                       ap=[[0, 128]] + list(src_ap.ap))

    with tile.TileContext(nc) as tc, ExitStack() as ctx:
        pers = ctx.enter_context(tc.tile_pool(name="pers", bufs=1))
        ps_pool = ctx.enter_context(tc.tile_pool(name="ps", bufs=2, space="PSUM"))
        esp = ctx.enter_context(tc.tile_pool(name="es", bufs=4))
        nrmp = ctx.enter_context(tc.tile_pool(name="nrm", bufs=2))
        yp = ctx.enter_context(tc.tile_pool(name="yp", bufs=4))

        # ---- persistent tiles ----
        wo_sb = pers.tile([128, 4, D], BF16)          # W_o rows, 4 chunks of 128
        bqk_sb = pers.tile([128, 8], F32)             # q|k bias per col-tile
        bv_bc = pers.tile([128, GD], F32)             # v bias bcast over tokens
        bo_bc = pers.tile([128, D], F32)              # out bias bcast over tokens
        bandneg = pers.tile([128, 2, 128], F32)       # 0 where col>=row else -1e30, x2
        qT_sb = pers.tile([128, 4, T], BF16)          # qT per head pair
        kT_sb = pers.tile([128, 4, T], BF16)          # kT per head pair (own tile:
                                                      # scores read lhsT from kT and
                                                      # rhs from qT concurrently)
        vaug = pers.tile([128, 2, 4, TK, 128], BF16)  # V_aug[par, hp, key chunk]
        xT_sb = pers.tile([128, 8 * T], BF16)         # packed (nt, d) blocks
        wq_sb = pers.tile([128, 12 * 1024], BF16)     # packed m/V blocks
        scr = pers.tile([128, 512], BF16)             # PE warmup scratch

        QKOFF = {0: 0, 4: 1024, 1: 6144, 5: 7168, 2: 8192, 6: 9216,
                 3: 10240, 7: 11264}
        VOFF = 2048

        def wq_qk(m, d8):
            return wq_sb[:, QKOFF[m] + 128 * d8:QKOFF[m] + 128 * (d8 + 1)]

        def xT_nt(nt, d8):
            return xT_sb[:, (nt * 8 + d8) * 512:(nt * 8 + d8) * 512 + 512]
        ctxn = pers.tile([128, 4, 4, 512], BF16)      # normalized ctx^T chunks

        # ---- PE warmup: matmuls on a zero scratch tile, starting as soon
        #      as the DVE memset lands (~8us), so the HAM clock is at 8/8 and
        #      the pipeline hot when the first real matmul's data arrives ----
        nc.vector.memset(scr[:], 0.0)
        for g in range(7):
            wps = ps_pool.tile([128, 512], F32, tag="sc", name=f"warm_{g}")
            for i in range(4):
                nc.tensor.matmul(wps[:], lhsT=scr[:, 0:128], rhs=scr[:],
                                 start=(i == 0), stop=(i == 3))

        # ---- loads: contiguous 1024-col chunks ordered by first use ----
        def chunks(sb, dram, lo, hi, step=1024):
            for a in range(lo, hi, step):
                nc.sync.dma_start(out=sb[:, a:a + step], in_=dram[:, a:a + step])

        chunks(wq_sb, wqkv_d, 0, 1024)          # m0
        chunks(xT_sb, xT_d, 0, 2048)            # nt0 d0..3
        chunks(wq_sb, wqkv_d, 1024, 2048)       # m4
        chunks(xT_sb, xT_d, 2048, 4096)         # nt0 d4..7
        nc.sync.dma_start(out=bqk_sb[:], in_=bqk_d)
        nc.sync.dma_start(out=bv_bc[:], in_=bcast128(bv_d))
        chunks(wq_sb, wqkv_d, 2048, 6144)       # V columns
        chunks(wq_sb, wqkv_d, 6144, 8192)       # m1 + m5 (pair-1 fillers read nt0)
        chunks(xT_sb, xT_d, 4096, 8192)         # nt1
        chunks(xT_sb, xT_d, 8192, 12288)        # nt2
        chunks(wq_sb, wqkv_d, 8192, 10240)      # m2 + m6
        chunks(xT_sb, xT_d, 12288, 16384)       # nt3
        chunks(wq_sb, wqkv_d, 10240, 12288)     # m3 + m7
        for c4 in range(4):
            nc.sync.dma_start(out=wo_sb[:, c4, :], in_=wo_d[128 * c4:128 * (c4 + 1), :])
        nc.sync.dma_start(out=bo_bc[:], in_=bcast128(bo_d))
        # bandneg[k, :, j] = 0 where j >= k else -1e30 (additive causal mask
        # for the 128 diagonal columns, applied on PSUM before exp; two copies
        # so both heads' regions mask with a single strided DVE add)
        nc.gpsimd.memset(bandneg[:], 0.0)
        for a in range(2):
            nc.gpsimd.affine_select(out=bandneg[:, a, :], in_=bandneg[:, a, :],
                                    compare_op=mybir.AluOpType.is_ge, fill=-1e30,
                                    base=0, pattern=[[1, 128]], channel_multiplier=-1)
        nc.gpsimd.memset(vaug[:, 0, :, :, 64:128], 1.0)   # even heads: [V | 1]
        nc.gpsimd.memset(vaug[:, 1, :, :, 0:64], 1.0)     # odd heads:  [1 | V]

        # ---- QKV projection pieces, emitted as PE fillers ----
        def qk_tile(m, nt):
            # k columns (m >= 4) beyond token 1792 are fully padded: never read
            w = 256 if (m >= 4 and nt == 3) else 512
            ps = ps_pool.tile([128, 512], F32, tag="p1", name=f"p1_{m}_{nt}")
            for d8 in range(8):
                nc.tensor.matmul(ps[:, 0:w], lhsT=wq_qk(m, d8),
                                 rhs=xT_nt(nt, d8)[:, 0:w],
                                 start=(d8 == 0), stop=(d8 == 7))
            dst = qT_sb if m < 4 else kT_sb
            nc.vector.tensor_scalar_add(dst[:, m % 4, 512 * nt:512 * nt + w],
                                        ps[:, 0:w], bqk_sb[:, m:m + 1])

        def v_tile(t16):
            ps = ps_pool.tile([128, 512], F32, tag="p1", name=f"p1v_{t16}")
            nt, to = t16 // 4, 128 * (t16 % 4)
            for d8 in range(8):
                nc.tensor.matmul(ps[:],
                                 lhsT=xT_sb[:, (nt * 8 + d8) * 512 + to:(nt * 8 + d8) * 512 + to + 128],
                                 rhs=wq_sb[:, VOFF + 512 * d8:VOFF + 512 * (d8 + 1)],
                                 start=(d8 == 0), stop=(d8 == 7))
            psv = ps.rearrange("p (hp par d) -> p hp par d", par=2, d=64)
            bvv = bv_bc.rearrange("p (hp par d) -> p hp par d", par=2, d=64)
            nc.vector.tensor_add(vaug[:, 0, :, t16, 0:64], psv[:, :, 0, :],
                                 bvv[:, :, 0, :])
            nc.vector.tensor_add(vaug[:, 1, :, t16, 64:128], psv[:, :, 1, :],
                                 bvv[:, :, 1, :])

        def av_pair(c, qt, kmax, cps, kb, est):
            off = max(0, 128 * kb - 512 * qt)
            w = 512 - off
            for par in (0, 1):
                nc.tensor.matmul(cps[par][:, off:512],
                                 lhsT=vaug[:, par, c, kb, :],
                                 rhs=est[:, 512 * par:512 * par + w],
                                 start=(kb == 0), stop=(kb == kmax))

        def attention_qt(c, qt, fillers=()):
            """Scores + exp + A@V for q-tile qt of head pair c.  The score
            pair is emitted back-to-back (concurrent row-tiled matmuls);
            A@V runs one key block behind so nothing splits the pair.
            `fillers` are independent PE work woven between key blocks to
            absorb the exp latency."""
            kmax = min(4 * qt + 3, TK - 1)
            fillers = list(fillers)
            nkb = kmax + 1
            fill_every = max(1, nkb // (len(fillers) + 1)) if fillers else 0
            cps = [ps_pool.tile([128, 512], F32, tag="cps", name=f"cps_{c}_{qt}_{i}")
                   for i in range(2)]
            prev = None
            for kb in range(nkb):
                if c == 0 and qt == kb // 4:   # JIT V chunks during pair 0
                    v_tile(kb)
                # diagonal blocks only need columns q >= 128*kb of the q-tile
                off = max(0, 128 * kb - 512 * qt)
                w = 512 - off
                psc = ps_pool.tile([128, 1024], F32, tag="sc", name=f"sc_{c}_{qt}_{kb}")
                for par in (0, 1):
                    r = 64 * par
                    nc.tensor.matmul(
                        psc[:, 512 * par:512 * par + w],
                        lhsT=kT_sb[r:r + 64, c, 128 * kb:128 * (kb + 1)],
                        rhs=qT_sb[r:r + 64, c, 512 * qt + off:512 * (qt + 1)],
                        start=True, stop=True)
                if kb >= 4 * qt:  # additive mask on the diagonal 128 columns
                    # one strided DVE add covers both heads' regions so the
                    # exp never waits on a second queued DVE op
                    pv = psc.rearrange("p (a b) -> p a b", a=2, b=512)
                    nc.vector.tensor_add(pv[:, :, 0:128], pv[:, :, 0:128],
                                         bandneg[:])
                est = esp.tile([128, 1024], BF16, tag="es", name=f"es_{c}_{qt}_{kb}")
                if w < 512:
                    # diagonal blocks: exp only the two live regions (one
                    # 2-region strided activation, skipping [w:512] garbage)
                    ev = est.rearrange("p (a b) -> p a b", a=2, b=512)
                    sv = psc.rearrange("p (a b) -> p a b", a=2, b=512)
                    nc.scalar.activation(ev[:, :, 0:w], sv[:, :, 0:w], AF.Exp,
                                         scale=float(1.0 / np.sqrt(HD)))
                else:
                    nc.scalar.activation(est[:], psc[:], AF.Exp,
                                         scale=float(1.0 / np.sqrt(HD)))
                if prev is not None:
                    av_pair(c, qt, kmax, cps, *prev)
                prev = (kb, est)
                if fillers and fill_every and kb % fill_every == fill_every - 1:
                    fillers.pop(0)()
            av_pair(c, qt, kmax, cps, *prev)
            for f in fillers:
                f()
            return cps

        def normalize(c, qt, cps):
            stg = nrmp.tile([128, 1024], F32, tag="stg", name=f"stg_{c}_{qt}")
            nc.vector.tensor_copy(stg[:, 0:512], cps[0][:])
            nc.vector.tensor_copy(stg[:, 512:1024], cps[1][:])
            # even head: ctx rows 0:64, sums rows 64:128 (V_aug = [V|1])
            # odd head:  sums rows 0:64, ctx rows 64:128 (V_aug = [1|V])
            # -> swap sums across partitions + gather ctx into one tile so a
            #    single full-width multiply normalizes both heads.  The final
            #    round's DMAs ride the Activation HWDGE ring, which is idle at
            #    the kernel tail (the sync ring is jammed with y stores).
            last = (c, qt) == (3, 3)
            eng = nc.scalar if last else nc.sync
            sums = nrmp.tile([128, 512], F32, tag="sums", name=f"sums_{c}_{qt}")
            eng.dma_start(out=sums[0:64, :], in_=stg[64:128, 0:512])
            eng.dma_start(out=sums[64:128, :], in_=stg[0:64, 512:1024])
            if last:
                # pipeline the tail: recip/multiply in 128-col chunks straight
                # from stg (no ctxc DMA on the critical chain) so the first
                # final-projection matmul starts ~2us earlier
                for j in range(2):
                    nc.vector.reciprocal_approx_fast(sums[:, 256 * j:256 * (j + 1)],
                                                     sums[:, 256 * j:256 * (j + 1)])
                    for i in range(2):
                        o = 256 * j + 128 * i
                        nc.vector.tensor_mul(ctxn[0:64, c, qt, o:o + 128],
                                             stg[0:64, o:o + 128],
                                             sums[0:64, o:o + 128])
                        nc.vector.tensor_mul(ctxn[64:128, c, qt, o:o + 128],
                                             stg[64:128, 512 + o:512 + o + 128],
                                             sums[64:128, o:o + 128])
            else:
                ctxc = nrmp.tile([128, 512], F32, tag="ctxc", name=f"ctxc_{c}_{qt}")
                eng.dma_start(out=ctxc[0:64, :], in_=stg[0:64, 0:512])
                eng.dma_start(out=ctxc[64:128, :], in_=stg[64:128, 512:1024])
                nc.vector.reciprocal_approx_fast(sums[:], sums[:])   # in place
                nc.vector.tensor_mul(ctxn[:, c, qt, :], ctxc[:], sums[:])

        def proj_part(t16, no, half):
            """Out-projection group for one [128 tok, 512] y half-tile.
            half None = all 4 head-pairs (+bias) -> ya.  For the last q-tile
            the group is split so most of it runs before the final attention
            round: half 0 = head-pairs 0,1 (+bias) -> ya; half 1 = pairs
            2,3 -> yb; the host sums the partials."""
            def emit():
                qt, o = t16 // 4, 128 * (t16 % 4)
                # tail groups cycle through all psum tags so their first
                # (pair-2) matmuls prefill while the final normalize chain
                # runs; attention tiles are done by then
                tag = ("p1", "sc", "cps")[(t16 * 2 + no) % 3] \
                    if (half == 1 and t16 >= 12) else "p1"
                ps = ps_pool.tile([128, 512], F32, tag=tag,
                                  name=f"yps_{t16}_{no}_{half}")
                cc = {None: (0, 1, 2, 3), 0: (0, 1), 1: (2, 3)}[half]
                for i, c4 in enumerate(cc):
                    nc.tensor.matmul(ps[:], lhsT=ctxn[:, c4, qt, o:o + 128],
                                     rhs=wo_sb[:, c4, 512 * no:512 * (no + 1)],
                                     start=(i == 0), stop=(i == len(cc) - 1))
                to_yb = half == 1 or (half is None and qt == 2)
                if to_yb:
                    yt = yp.tile([128, 512], BF16, tag="yh", name=f"yh_{t16}_{no}")
                else:
                    yt = yp.tile([128, 512], F32, tag="y", name=f"y_{t16}_{no}_{half}")
                if half == 1:
                    nc.vector.tensor_copy(yt[:], ps[:])
                else:
                    nc.vector.tensor_add(yt[:], ps[:], bo_bc[:, 512 * no:512 * (no + 1)])
                yd = yb_d if to_yb else ya_d
                nc.sync.dma_start(out=yd[128 * t16:128 * (t16 + 1), 512 * no:512 * (no + 1)],
                                  in_=yt[:])
            return emit

        # ---- interleaved schedule ----
        for c in range(4):
            for qt in range(4):
                if c == 0:
                    qk_tile(0, qt)
                    qk_tile(4, qt)
                    fillers = [lambda n=qt: qk_tile(1, n),
                               lambda n=qt: qk_tile(5, n)]
                elif c == 1:
                    fillers = [lambda n=qt: qk_tile(2, n),
                               lambda n=qt: qk_tile(6, n)]
                elif c == 2:
                    # pair-3 qk (prereqs for pair-3 rounds) + the last
                    # q-tile's first half-proj (ctxn(0..1, 3) is complete
                    # once pair 1 finished)
                    fillers = [lambda n=qt: qk_tile(3, n),
                               lambda n=qt: qk_tile(7, n)]
                    fillers += [proj_part(11 + qt, no, 0) for no in range(2)] \
                        if qt > 0 else []
                else:
                    if qt == 0:
                        fillers = [proj_part(15, no, 0) for no in range(2)]
                    elif qt == 1:
                        fillers = [proj_part(t16, no, None)
                                   for t16 in range(0, 4) for no in range(2)]
                    else:
                        fillers = [proj_part(t16, no, None)
                                   for t16 in range(4 * (qt - 1), 4 * qt)
                                   for no in range(2)]
                cps = attention_qt(c, qt, fillers)
                normalize(c, qt, cps)
        for t16 in range(12, 16):   # tail: last queries' second half-proj
            for no in range(2):
                proj_part(t16, no, 1)()

    nc.compile()
    return nc


def _reference_np(x, W_qkv, b_qkv, W_o, b_o, key_padding_mask):
    """Numpy fallback for inputs that do not match the compiled assumptions."""
    b_, t_, d_ = x.shape
    hd = d_ // H
    qkv = x.astype(np.float64) @ W_qkv.astype(np.float64) + b_qkv
    q, k, v = np.split(qkv, 3, axis=-1)

    def heads(t):
        return t.reshape(b_, t_, H, hd).transpose(0, 2, 1, 3)

    q, k, v = heads(q), heads(k), heads(v)
    s = np.einsum("bhqd,bhkd->bhqk", q, k) / np.sqrt(hd)
    causal = np.triu(np.ones((t_, t_), bool), k=1)
    mask = key_padding_mask[:, None, None, :] | causal[None, None]
    s = np.where(mask, -np.inf, s)
    s = s - s.max(axis=-1, keepdims=True)
    e = np.exp(s)
    with np.errstate(invalid="ignore"):
        a = e / e.sum(axis=-1, keepdims=True)
    ctx = np.einsum("bhqk,bhkd->bhqd", a, v)
    y = ctx.transpose(0, 2, 1, 3).reshape(b_, t_, d_) @ W_o.astype(np.float64) + b_o
    return y.astype(np.float32)


def kernel(x, W_qkv, b_qkv, W_o, b_o, key_padding_mask):
    x = np.asarray(x)
    W_qkv, b_qkv = np.asarray(W_qkv), np.asarray(b_qkv)
    W_o, b_o = np.asarray(W_o), np.asarray(b_o)
    key_padding_mask = np.asarray(key_padding_mask)

    expected_mask = np.zeros((B, T), bool)
    expected_mask[:, T - NPAD:] = True
    if (x.shape != (B, T, D) or not np.array_equal(key_padding_mask, expected_mask)):
        return _reference_np(x, W_qkv, b_qkv, W_o, b_o, key_padding_mask)

    if "nc" not in _CACHE:
        _CACHE["nc"] = _build()
    nc = _CACHE["nc"]

    bf = ml_dtypes.bfloat16
    in_maps = []
    for c in range(N_CORES):
        b, g = divmod(c, 2)
        cols = slice(g * GD, (g + 1) * GD)
        wq = np.concatenate([W_qkv[:, cols], W_qkv[:, D + g * GD:D + (g + 1) * GD],
                             W_qkv[:, 2 * D + g * GD:2 * D + (g + 1) * GD]],
                            axis=1).astype(bf)
        bq = np.concatenate([b_qkv[cols], b_qkv[D + g * GD:D + (g + 1) * GD]])
        xT = np.ascontiguousarray(x[b].T).astype(bf)
        # pack wq columns: m0 | m4 | V | m1 m5 m2 m6 m3 m7 (d-major inside)
        wq_blocks = []
        for m in (0, 4):
            wq_blocks += [wq[128 * d:128 * (d + 1), 128 * m:128 * (m + 1)]
                          for d in range(8)]
        wq_blocks += [wq[128 * d:128 * (d + 1), 1024:1536] for d in range(8)]
        for m in (1, 5, 2, 6, 3, 7):
            wq_blocks += [wq[128 * d:128 * (d + 1), 128 * m:128 * (m + 1)]
                          for d in range(8)]
        wq_p = np.concatenate(wq_blocks, axis=1)
        # pack xT columns: (nt, d) blocks of 512 tokens
        xT_p = np.concatenate([xT[128 * d:128 * (d + 1), 512 * nt:512 * (nt + 1)]
                               for nt in range(4) for d in range(8)], axis=1)
        in_maps.append({
            "xT": np.ascontiguousarray(xT_p),
            "wqkv": np.ascontiguousarray(wq_p),
            "wo": np.ascontiguousarray(W_o[g * GD:(g + 1) * GD, :]).astype(bf),
            "bqk": np.ascontiguousarray(bq.reshape(8, 128).T.astype(np.float32)),
            "bv": np.ascontiguousarray(b_qkv[2 * D + g * GD:2 * D + (g + 1) * GD]).astype(np.float32),
            "bo": np.ascontiguousarray(0.5 * b_o).astype(np.float32),
        })

    trace = bool(os.environ.get("MHA_TRACE"))
    if trace:
        _register_ntff_hook()
    res = run_bass_kernel_spmd(nc, in_maps, core_ids=list(range(N_CORES)),
                               trace=trace)
    if trace:
        _CACHE["exec_time_ns"] = res.exec_time_ns

    y = np.empty((B, T, D), np.float32)
    for b in range(B):
        ya0, ya1 = res.results[2 * b]["ya"], res.results[2 * b + 1]["ya"]
        yb0 = res.results[2 * b]["yb"].astype(np.float32)
        yb1 = res.results[2 * b + 1]["yb"].astype(np.float32)
        y[b] = ya0 + ya1
        y[b, 1024:1536] = yb0[1024:1536] + yb1[1024:1536]  # qt2 rows ride yb
        y[b, 1536:] += yb0[1536:] + yb1[1536:]             # pairs-2,3 partials
    return y


def _register_ntff_hook():
    """antenv.axon_hooks is absent in this container; synthesize it so
    run_bass_kernel_spmd(trace=True) can NTFF-profile via ctypes."""
    import types

    if "antenv.axon_hooks" in sys.modules:
        return
    sys.path.insert(0, "/root/.axon_site")
    from trn_agent_boot.trn_boot import _ntff_profile_via_ctypes

    hook = _ntff_profile_via_ctypes("/opt/axon/libaxon_pjrt.so")
    mod = types.ModuleType("antenv.axon_hooks")
    mod._hook = hook
    mod.get_axon_ntff_profile_hook = lambda: mod._hook
    mod.set_axon_ntff_profile_hook = lambda h: setattr(mod, "_hook", h)
    sys.modules["antenv.axon_hooks"] = mod


# revision 29
# speedup vs baseline: 1.1844x; 1.0021x over previous
"""Masked multi-head attention (B=4, T=2048, D=1024, H=16) on 8 trn2 NeuronCores.

Sharding: core c handles batch b = c//2 and head-group g = c%2 (8 heads, 512
of the 1024 model dims).  Each core runs the fused QKV projection for its
head-group over its batch, causal+padding-masked attention for its 8 heads,
and a partial out-projection (its 512 rows of W_o).  Device emits two partial
y tensors (head-pair halves); the host sums the four partials per batch.

Device algorithm (per core), all matmuls bf16 with f32 PSUM accumulation:
  - qT,kT  = (x @ Wq|k)^T computed directly in [dims, tok] layout
             (lhsT = W chunk, rhs = xT chunk), bias added per-partition.
  - V      computed in natural [tok, dims] layout, packed into
             V_aug = [V | 1] (even heads) or [1 | V] (odd heads) so A@V_aug
             also yields the softmax row-sums replicated across 64 partitions.
  - scores S^T[k, q] per 128-key block kb: the two heads of a pair use PE
             row groups 0:64 / 64:128 and separate PSUM banks, so the pair
             of score matmuls runs CONCURRENTLY in the array when adjacent
             in the PE stream (row-tiled).  Keys >= 1792 (padded) never
             computed; causal handled by skipping blocks + an additive
             -1e30 mask on the 128 diagonal columns (DVE add on PSUM before
             exp, keeping the exp->A@V path short).
  - ctx^T  accumulated over key blocks in PSUM; A@V pipelined one key-block
             behind scores so score pairs stay adjacent (concurrent).
  - y      = ctx @ W_o rows in two head-pair halves (partA: c4 0,1 with
             bias; partB: c4 2,3) -> separate DRAM outputs ya/yb summed on
             host.  partA runs as PE filler during pair-2 attention, partB
             during pair-3; only the last 512 queries' partB remains as tail.

Scheduling: Tile's priority scheduler picks the lowest-priority READY PE
instruction; emission order sets priority.  Emitting scores(kb) before
A@V(kb-1) keeps score pairs adjacent; qk/v/proj tiles are woven as fillers
so the PE never idles while ScalarE exps (1.1us each) run.  A scratch-tile
warmup burst keeps PE busy from ~8us (HAM warm) while inputs DMA in
1024-column chunks ordered by first use.
"""

import os
import sys

sys.path.insert(0, "/opt/trn_rl_repo")

from contextlib import ExitStack

import ml_dtypes
import numpy as np

import concourse.bass as bass
import concourse.tile as tile
from concourse import bacc, mybir
from concourse.bass_utils import run_bass_kernel_spmd

B, T, D, H, HD = 4, 2048, 1024, 16, 64
N_CORES = 8
NH = H // 2            # heads per core = 8
GD = NH * HD           # head-group width = 512
TK = 14                # valid 128-key blocks (keys < 1792; rest padded)
NPAD = 256             # padded key positions at the end
BF16 = mybir.dt.bfloat16
F32 = mybir.dt.float32
AF = mybir.ActivationFunctionType

_CACHE = {}


def _build():
    nc = bacc.Bacc("TRN2", target_bir_lowering=False, debug=False,
                   num_devices=N_CORES)
    # xT packed as [128, (nt, d) blocks of 512]; wq packed as
    # [128, m0|m4|V|m1|m5|m2|m6|m3|m7 blocks] -- both host-reordered so every
    # DMA chunk is fully contiguous (large descriptors, ordered by first use).
    xT_d = nc.dram_tensor("xT", [128, 8 * T], BF16, kind="ExternalInput").ap()
    wqkv_d = nc.dram_tensor("wqkv", [128, 12 * 1024], BF16,
                            kind="ExternalInput").ap()
    wo_d = nc.dram_tensor("wo", [GD, D], BF16, kind="ExternalInput").ap()
    bqk_d = nc.dram_tensor("bqk", [128, 8], F32, kind="ExternalInput").ap()
    bv_d = nc.dram_tensor("bv", [GD], F32, kind="ExternalInput").ap()
    bo_d = nc.dram_tensor("bo", [D], F32, kind="ExternalInput").ap()
    ya_d = nc.dram_tensor("ya", [T, D], F32, kind="ExternalOutput").ap()
    # pairs-2,3 partial for the last 512 queries; bf16 partials halve the
    # tail DMA (host adds in f32)
    yb_d = nc.dram_tensor("yb", [T, D], BF16, kind="ExternalOutput").ap()

    def bcast128(src_ap):
        """DMA access pattern replicating a 1-D dram vector over 128 partitions."""
        return bass.AP(tensor=src_ap.tensor, offset=src_ap.offset,
                       ap=[[0, 128]] + list(src_ap.ap))

    with tile.TileContext(nc) as tc, ExitStack() as ctx:
        pers = ctx.enter_context(tc.tile_pool(name="pers", bufs=1))
        ps_pool = ctx.enter_context(tc.tile_pool(name="ps", bufs=2, space="PSUM"))
        esp = ctx.enter_context(tc.tile_pool(name="es", bufs=4))
        nrmp = ctx.enter_context(tc.tile_pool(name="nrm", bufs=2))
        yp = ctx.enter_context(tc.tile_pool(name="yp", bufs=4))

        # ---- persistent tiles ----
        wo_sb = pers.tile([128, 4, D], BF16)          # W_o rows, 4 chunks of 128
        bqk_sb = pers.tile([128, 8], F32)             # q|k bias per col-tile
        bv_bc = pers.tile([128, GD], F32)             # v bias bcast over tokens
        bo_bc = pers.tile([128, D], F32)              # out bias bcast over tokens
        bandneg = pers.tile([128, 2, 128], F32)       # 0 where col>=row else -1e30, x2
        qT_sb = pers.tile([128, 4, T], BF16)          # qT per head pair
        kT_sb = pers.tile([128, 4, T], BF16)          # kT per head pair (own tile:
                                                      # scores read lhsT from kT and
                                                      # rhs from qT concurrently)
        vaug = pers.tile([128, 2, 4, TK, 128], BF16)  # V_aug[par, hp, key chunk]
        xT_sb = pers.tile([128, 8 * T], BF16)         # packed (nt, d) blocks
        wq_sb = pers.tile([128, 12 * 1024], BF16)     # packed m/V blocks
        scr = pers.tile([128, 512], BF16)             # PE warmup scratch

        QKOFF = {0: 0, 4: 1024, 1: 6144, 5: 7168, 2: 8192, 6: 9216,
                 3: 10240, 7: 11264}
        VOFF = 2048

        def wq_qk(m, d8):
            return wq_sb[:, QKOFF[m] + 128 * d8:QKOFF[m] + 128 * (d8 + 1)]

        def xT_nt(nt, d8):
            return xT_sb[:, (nt * 8 + d8) * 512:(nt * 8 + d8) * 512 + 512]
        ctxn = pers.tile([128, 4, 4, 512], BF16)      # normalized ctx^T chunks

        # ---- PE warmup: matmuls on a zero scratch tile, starting as soon
        #      as the DVE memset lands (~8us), so the HAM clock is at 8/8 and
        #      the pipeline hot when the first real matmul's data arrives ----
        nc.vector.memset(scr[:], 0.0)
        for g in range(7):
            wps = ps_pool.tile([128, 512], F32, tag="sc", name=f"warm_{g}")
            for i in range(4):
                nc.tensor.matmul(wps[:], lhsT=scr[:, 0:128], rhs=scr[:],
                                 start=(i == 0), stop=(i == 3))

        # ---- loads: contiguous 1024-col chunks ordered by first use ----
        def chunks(sb, dram, lo, hi, step=1024):
            for a in range(lo, hi, step):
                nc.sync.dma_start(out=sb[:, a:a + step], in_=dram[:, a:a + step])

        chunks(wq_sb, wqkv_d, 0, 1024)          # m0
        chunks(xT_sb, xT_d, 0, 2048)            # nt0 d0..3
        chunks(wq_sb, wqkv_d, 1024, 2048)       # m4
        chunks(xT_sb, xT_d, 2048, 4096)         # nt0 d4..7
        nc.sync.dma_start(out=bqk_sb[:], in_=bqk_d)
        nc.sync.dma_start(out=bv_bc[:], in_=bcast128(bv_d))
        chunks(wq_sb, wqkv_d, 2048, 6144)       # V columns
        chunks(wq_sb, wqkv_d, 6144, 8192)       # m1 + m5 (pair-1 fillers read nt0)
        chunks(xT_sb, xT_d, 4096, 8192)         # nt1
        chunks(xT_sb, xT_d, 8192, 12288)        # nt2
        chunks(wq_sb, wqkv_d, 8192, 10240)      # m2 + m6
        chunks(xT_sb, xT_d, 12288, 16384)       # nt3
        chunks(wq_sb, wqkv_d, 10240, 12288)     # m3 + m7
        for c4 in range(4):
            nc.sync.dma_start(out=wo_sb[:, c4, :], in_=wo_d[128 * c4:128 * (c4 + 1), :])
        nc.sync.dma_start(out=bo_bc[:], in_=bcast128(bo_d))
        # bandneg[k, :, j] = 0 where j >= k else -1e30 (additive causal mask
        # for the 128 diagonal columns, applied on PSUM before exp; two copies
        # so both heads' regions mask with a single strided DVE add)
        nc.gpsimd.memset(bandneg[:], 0.0)
        for a in range(2):
            nc.gpsimd.affine_select(out=bandneg[:, a, :], in_=bandneg[:, a, :],
                                    compare_op=mybir.AluOpType.is_ge, fill=-1e30,
                                    base=0, pattern=[[1, 128]], channel_multiplier=-1)
        nc.gpsimd.memset(vaug[:, 0, :, :, 64:128], 1.0)   # even heads: [V | 1]
        nc.gpsimd.memset(vaug[:, 1, :, :, 0:64], 1.0)     # odd heads:  [1 | V]

        # ---- QKV projection pieces, emitted as PE fillers ----
        def qk_tile(m, nt):
            # k columns (m >= 4) beyond token 1792 are fully padded: never read
            w = 256 if (m >= 4 and nt == 3) else 512
            ps = ps_pool.tile([128, 512], F32, tag="p1", name=f"p1_{m}_{nt}")
            for d8 in range(8):
                nc.tensor.matmul(ps[:, 0:w], lhsT=wq_qk(m, d8),
                                 rhs=xT_nt(nt, d8)[:, 0:w],
                                 start=(d8 == 0), stop=(d8 == 7))
            dst = qT_sb if m < 4 else kT_sb
            nc.vector.tensor_scalar_add(dst[:, m % 4, 512 * nt:512 * nt + w],
                                        ps[:, 0:w], bqk_sb[:, m:m + 1])

        def v_tile(t16):
            ps = ps_pool.tile([128, 512], F32, tag="p1", name=f"p1v_{t16}")
            nt, to = t16 // 4, 128 * (t16 % 4)
            for d8 in range(8):
                nc.tensor.matmul(ps[:],
                                 lhsT=xT_sb[:, (nt * 8 + d8) * 512 + to:(nt * 8 + d8) * 512 + to + 128],
                                 rhs=wq_sb[:, VOFF + 512 * d8:VOFF + 512 * (d8 + 1)],
                                 start=(d8 == 0), stop=(d8 == 7))
            psv = ps.rearrange("p (hp par d) -> p hp par d", par=2, d=64)
            bvv = bv_bc.rearrange("p (hp par d) -> p hp par d", par=2, d=64)
            nc.vector.tensor_add(vaug[:, 0, :, t16, 0:64], psv[:, :, 0, :],
                                 bvv[:, :, 0, :])
            nc.vector.tensor_add(vaug[:, 1, :, t16, 64:128], psv[:, :, 1, :],
                                 bvv[:, :, 1, :])

        def av_pair(c, qt, kmax, cps, kb, est):
            off = max(0, 128 * kb - 512 * qt)
            w = 512 - off
            for par in (0, 1):
                nc.tensor.matmul(cps[par][:, off:512],
                                 lhsT=vaug[:, par, c, kb, :],
                                 rhs=est[:, 512 * par:512 * par + w],
                                 start=(kb == 0), stop=(kb == kmax))

        def attention_qt(c, qt, fillers=()):
            """Scores + exp + A@V for q-tile qt of head pair c.  The score
            pair is emitted back-to-back (concurrent row-tiled matmuls);
            A@V runs one key block behind so nothing splits the pair.
            `fillers` are independent PE work woven between key blocks to
            absorb the exp latency."""
            kmax = min(4 * qt + 3, TK - 1)
            fillers = list(fillers)
            nkb = kmax + 1
            fill_every = max(1, nkb // (len(fillers) + 1)) if fillers else 0
            cps = [ps_pool.tile([128, 512], F32, tag="cps", name=f"cps_{c}_{qt}_{i}")
                   for i in range(2)]
            prev = None
            for kb in range(nkb):
                if c == 0 and qt == kb // 4:   # JIT V chunks during pair 0
                    v_tile(kb)
                # diagonal blocks only need columns q >= 128*kb of the q-tile
                off = max(0, 128 * kb - 512 * qt)
                w = 512 - off
                psc = ps_pool.tile([128, 1024], F32, tag="sc", name=f"sc_{c}_{qt}_{kb}")
                for par in (0, 1):
                    r = 64 * par
                    nc.tensor.matmul(
                        psc[:, 512 * par:512 * par + w],
                        lhsT=kT_sb[r:r + 64, c, 128 * kb:128 * (kb + 1)],
                        rhs=qT_sb[r:r + 64, c, 512 * qt + off:512 * (qt + 1)],
                        start=True, stop=True)
                if kb >= 4 * qt:  # additive mask on the diagonal 128 columns
                    # one strided DVE add covers both heads' regions so the
                    # exp never waits on a second queued DVE op
                    pv = psc.rearrange("p (a b) -> p a b", a=2, b=512)
                    nc.vector.tensor_add(pv[:, :, 0:128], pv[:, :, 0:128],
                                         bandneg[:])
                est = esp.tile([128, 1024], BF16, tag="es", name=f"es_{c}_{qt}_{kb}")
                if w < 512:
                    # diagonal blocks: exp only the two live regions (one
                    # 2-region strided activation, skipping [w:512] garbage)
                    ev = est.rearrange("p (a b) -> p a b", a=2, b=512)
                    sv = psc.rearrange("p (a b) -> p a b", a=2, b=512)
                    nc.scalar.activation(ev[:, :, 0:w], sv[:, :, 0:w], AF.Exp,
                                         scale=float(1.0 / np.sqrt(HD)))
                else:
                    nc.scalar.activation(est[:], psc[:], AF.Exp,
                                         scale=float(1.0 / np.sqrt(HD)))
                if prev is not None:
                    av_pair(c, qt, kmax, cps, *prev)
                prev = (kb, est)
                if fillers and fill_every and kb % fill_every == fill_every - 1:
                    fillers.pop(0)()
            av_pair(c, qt, kmax, cps, *prev)
            for f in fillers:
                f()
            return cps

        def normalize(c, qt, cps):
            stg = nrmp.tile([128, 1024], F32, tag="stg", name=f"stg_{c}_{qt}")
            nc.vector.tensor_copy(stg[:, 0:512], cps[0][:])
            nc.vector.tensor_copy(stg[:, 512:1024], cps[1][:])
            # even head: ctx rows 0:64, sums rows 64:128 (V_aug = [V|1])
            # odd head:  sums rows 0:64, ctx rows 64:128 (V_aug = [1|V])
            # -> swap sums across partitions + gather ctx into one tile so a
            #    single full-width multiply normalizes both heads.  The final
            #    round's DMAs ride the Activation HWDGE ring, which is idle at
            #    the kernel tail (the sync ring is jammed with y stores).
            last = (c, qt) == (3, 3)
            eng = nc.scalar if last else nc.sync
            sums = nrmp.tile([128, 512], F32, tag="sums", name=f"sums_{c}_{qt}")
            eng.dma_start(out=sums[0:64, :], in_=stg[64:128, 0:512])
            eng.dma_start(out=sums[64:128, :], in_=stg[0:64, 512:1024])
            if last:
                # pipeline the tail: recip/multiply in 128-col chunks straight
                # from stg (no ctxc DMA on the critical chain) so the first
                # final-projection matmul starts ~2us earlier
                for j in range(2):
                    nc.vector.reciprocal_approx_fast(sums[:, 256 * j:256 * (j + 1)],
                                                     sums[:, 256 * j:256 * (j + 1)])
                    for i in range(2):
                        o = 256 * j + 128 * i
                        nc.vector.tensor_mul(ctxn[0:64, c, qt, o:o + 128],
                                             stg[0:64, o:o + 128],
                                             sums[0:64, o:o + 128])
                        nc.vector.tensor_mul(ctxn[64:128, c, qt, o:o + 128],
                                             stg[64:128, 512 + o:512 + o + 128],
                                             sums[64:128, o:o + 128])
            else:
                ctxc = nrmp.tile([128, 512], F32, tag="ctxc", name=f"ctxc_{c}_{qt}")
                eng.dma_start(out=ctxc[0:64, :], in_=stg[0:64, 0:512])
                eng.dma_start(out=ctxc[64:128, :], in_=stg[64:128, 512:1024])
                nc.vector.reciprocal_approx_fast(sums[:], sums[:])   # in place
                nc.vector.tensor_mul(ctxn[:, c, qt, :], ctxc[:], sums[:])

        def proj_part(t16, no, half):
            """Out-projection group for one [128 tok, 512] y half-tile.
            half None = all 4 head-pairs (+bias) -> ya.  For the last q-tile
            the group is split so most of it runs before the final attention
            round: half 0 = head-pairs 0,1 (+bias) -> ya; half 1 = pairs
            2,3 -> yb; the host sums the partials."""
            def emit():
                qt, o = t16 // 4, 128 * (t16 % 4)
                # tail groups cycle through all psum tags so their first
                # (pair-2) matmuls prefill while the final normalize chain
                # runs; attention tiles are done by then
                tag = ("p1", "sc", "cps")[(t16 * 2 + no) % 3] \
                    if (half == 1 and t16 >= 12) else "p1"
                ps = ps_pool.tile([128, 512], F32, tag=tag,
                                  name=f"yps_{t16}_{no}_{half}")
                cc = {None: (0, 1, 2, 3), 0: (0, 1), 1: (2, 3)}[half]
                for i, c4 in enumerate(cc):
                    nc.tensor.matmul(ps[:], lhsT=ctxn[:, c4, qt, o:o + 128],
                                     rhs=wo_sb[:, c4, 512 * no:512 * (no + 1)],
                                     start=(i == 0), stop=(i == len(cc) - 1))
                to_yb = half == 1 or (half is None and qt == 2)
                if to_yb:
                    yt = yp.tile([128, 512], BF16, tag="yh", name=f"yh_{t16}_{no}")
                else:
                    yt = yp.tile([128, 512], F32, tag="y", name=f"y_{t16}_{no}_{half}")
                if half == 1:
                    nc.vector.tensor_copy(yt[:], ps[:])
                else:
                    nc.vector.tensor_add(yt[:], ps[:], bo_bc[:, 512 * no:512 * (no + 1)])
                yd = yb_d if to_yb else ya_d
                nc.sync.dma_start(out=yd[128 * t16:128 * (t16 + 1), 512 * no:512 * (no + 1)],
                                  in_=yt[:])
            return emit

        # ---- interleaved schedule ----
        for c in range(4):
            for qt in range(4):
                if c == 0:
                    qk_tile(0, qt)
                    qk_tile(4, qt)
                    fillers = [lambda n=qt: qk_tile(1, n),
                               lambda n=qt: qk_tile(5, n)]
                elif c == 1:
                    fillers = [lambda n=qt: qk_tile(2, n),
                               lambda n=qt: qk_tile(6, n)]
                elif c == 2:
                    # pair-3 qk (prereqs for pair-3 rounds) + the last
                    # q-tile's first half-proj (ctxn(0..1, 3) is complete
                    # once pair 1 finished)
                    fillers = [lambda n=qt: qk_tile(3, n),
                               lambda n=qt: qk_tile(7, n)]
                    fillers += [proj_part(11 + qt, no, 0) for no in range(2)] \
                        if qt > 0 else []
                else:
                    if qt == 0:
                        fillers = [proj_part(15, no, 0) for no in range(2)]
                    elif qt == 1:
                        fillers = [proj_part(t16, no, None)
                                   for t16 in range(0, 4) for no in range(2)]
                    else:
                        fillers = [proj_part(t16, no, None)
                                   for t16 in range(4 * (qt - 1), 4 * qt)
                                   for no in range(2)]
                cps = attention_qt(c, qt, fillers)
                normalize(c, qt, cps)
        for t16 in range(12, 16):   # tail: last queries' second half-proj
            for no in range(2):
                proj_part(t16, no, 1)()

    nc.compile()
    return nc


def _reference_np(x, W_qkv, b_qkv, W_o, b_o, key_padding_mask):
    """Numpy fallback for inputs that do not match the compiled assumptions."""
    b_, t_, d_ = x.shape
    hd = d_ // H
    qkv = x.astype(np.float64) @ W_qkv.astype(np.float64) + b_qkv
    q, k, v = np.split(qkv, 3, axis=-1)

    def heads(t):
        return t.reshape(b_, t_, H, hd).transpose(0, 2, 1, 3)

    q, k, v = heads(q), heads(k), heads(v)
    s = np.einsum("bhqd,bhkd->bhqk", q, k) / np.sqrt(hd)
    causal = np.triu(np.ones((t_, t_), bool), k=1)
    mask = key_padding_mask[:, None, None, :] | causal[None, None]
    s = np.where(mask, -np.inf, s)
    s = s - s.max(axis=-1, keepdims=True)
    e = np.exp(s)
    with np.errstate(invalid="ignore"):
        a = e / e.sum(axis=-1, keepdims=True)
    ctx = np.einsum("bhqk,bhkd->bhqd", a, v)
    y = ctx.transpose(0, 2, 1, 3).reshape(b_, t_, d_) @ W_o.astype(np.float64) + b_o
    return y.astype(np.float32)


def kernel(x, W_qkv, b_qkv, W_o, b_o, key_padding_mask):
    x = np.asarray(x)
    W_qkv, b_qkv = np.asarray(W_qkv), np.asarray(b_qkv)
    W_o, b_o = np.asarray(W_o), np.asarray(b_o)
    key_padding_mask = np.asarray(key_padding_mask)

    expected_mask = np.zeros((B, T), bool)
    expected_mask[:, T - NPAD:] = True
    if (x.shape != (B, T, D) or not np.array_equal(key_padding_mask, expected_mask)):
        return _reference_np(x, W_qkv, b_qkv, W_o, b_o, key_padding_mask)

    if "nc" not in _CACHE:
        _CACHE["nc"] = _build()
    nc = _CACHE["nc"]

    bf = ml_dtypes.bfloat16
    in_maps = []
    for c in range(N_CORES):
        b, g = divmod(c, 2)
        cols = slice(g * GD, (g + 1) * GD)
        wq = np.concatenate([W_qkv[:, cols], W_qkv[:, D + g * GD:D + (g + 1) * GD],
                             W_qkv[:, 2 * D + g * GD:2 * D + (g + 1) * GD]],
                            axis=1).astype(bf)
        bq = np.concatenate([b_qkv[cols], b_qkv[D + g * GD:D + (g + 1) * GD]])
        xT = np.ascontiguousarray(x[b].T).astype(bf)
        # pack wq columns: m0 | m4 | V | m1 m5 m2 m6 m3 m7 (d-major inside)
        wq_blocks = []
        for m in (0, 4):
            wq_blocks += [wq[128 * d:128 * (d + 1), 128 * m:128 * (m + 1)]
                          for d in range(8)]
        wq_blocks += [wq[128 * d:128 * (d + 1), 1024:1536] for d in range(8)]
        for m in (1, 5, 2, 6, 3, 7):
            wq_blocks += [wq[128 * d:128 * (d + 1), 128 * m:128 * (m + 1)]
                          for d in range(8)]
        wq_p = np.concatenate(wq_blocks, axis=1)
        # pack xT columns: (nt, d) blocks of 512 tokens
        xT_p = np.concatenate([xT[128 * d:128 * (d + 1), 512 * nt:512 * (nt + 1)]
                               for nt in range(4) for d in range(8)], axis=1)
        in_maps.append({
            "xT": np.ascontiguousarray(xT_p),
            "wqkv": np.ascontiguousarray(wq_p),
            "wo": np.ascontiguousarray(W_o[g * GD:(g + 1) * GD, :]).astype(bf),
            "bqk": np.ascontiguousarray(bq.reshape(8, 128).T.astype(np.float32)),
            "bv": np.ascontiguousarray(b_qkv[2 * D + g * GD:2 * D + (g + 1) * GD]).astype(np.float32),
            "bo": np.ascontiguousarray(0.5 * b_o).astype(np.float32),
        })

    trace = bool(os.environ.get("MHA_TRACE"))
    if trace:
        _register_ntff_hook()
    res = run_bass_kernel_spmd(nc, in_maps, core_ids=list(range(N_CORES)),
                               trace=trace)
    if trace:
        _CACHE["exec_time_ns"] = res.exec_time_ns

    y = np.empty((B, T, D), np.float32)
    for b in range(B):
        ya0, ya1 = res.results[2 * b]["ya"], res.results[2 * b + 1]["ya"]
        yb0 = res.results[2 * b]["yb"].astype(np.float32)
        yb1 = res.results[2 * b + 1]["yb"].astype(np.float32)
        y[b] = ya0 + ya1
        y[b, 1024:1536] = yb0[1024:1536] + yb1[1024:1536]  # qt2 rows ride yb
        y[b, 1536:] += yb0[1536:] + yb1[1536:]             # pairs-2,3 partials
    return y


def _register_ntff_hook():
    """antenv.axon_hooks is absent in this container; synthesize it so
    run_bass_kernel_spmd(trace=True) can NTFF-profile via ctypes."""
    import types

    if "antenv.axon_hooks" in sys.modules:
        return
    sys.path.insert(0, "/root/.axon_site")
    from trn_agent_boot.trn_boot import _ntff_profile_via_ctypes

    hook = _ntff_profile_via_ctypes("/opt/axon/libaxon_pjrt.so")
    mod = types.ModuleType("antenv.axon_hooks")
    mod._hook = hook
    mod.get_axon_ntff_profile_hook = lambda: mod._hook
    mod.set_axon_ntff_profile_hook = lambda h: setattr(mod, "_hook", h)
    sys.modules["antenv.axon_hooks"] = mod


# revision 30
# speedup vs baseline: 1.1905x; 1.0052x over previous
"""Masked multi-head attention (B=4, T=2048, D=1024, H=16) on 8 trn2 NeuronCores.

Sharding: core c handles batch b = c//2 and head-group g = c%2 (8 heads, 512
of the 1024 model dims).  Each core runs the fused QKV projection for its
head-group over its batch, causal+padding-masked attention for its 8 heads,
and a partial out-projection (its 512 rows of W_o).  Device emits two partial
y tensors (head-pair halves); the host sums the four partials per batch.

Device algorithm (per core), all matmuls bf16 with f32 PSUM accumulation:
  - qT,kT  = (x @ Wq|k)^T computed directly in [dims, tok] layout
             (lhsT = W chunk, rhs = xT chunk), bias added per-partition.
  - V      computed in natural [tok, dims] layout, packed into
             V_aug = [V | 1] (even heads) or [1 | V] (odd heads) so A@V_aug
             also yields the softmax row-sums replicated across 64 partitions.
  - scores S^T[k, q] per 128-key block kb: the two heads of a pair use PE
             row groups 0:64 / 64:128 and separate PSUM banks, so the pair
             of score matmuls runs CONCURRENTLY in the array when adjacent
             in the PE stream (row-tiled).  Keys >= 1792 (padded) never
             computed; causal handled by skipping blocks + an additive
             -1e30 mask on the 128 diagonal columns (DVE add on PSUM before
             exp, keeping the exp->A@V path short).
  - ctx^T  accumulated over key blocks in PSUM; A@V pipelined one key-block
             behind scores so score pairs stay adjacent (concurrent).
  - y      = ctx @ W_o rows in two head-pair halves (partA: c4 0,1 with
             bias; partB: c4 2,3) -> separate DRAM outputs ya/yb summed on
             host.  partA runs as PE filler during pair-2 attention, partB
             during pair-3; only the last 512 queries' partB remains as tail.

Scheduling: Tile's priority scheduler picks the lowest-priority READY PE
instruction; emission order sets priority.  Emitting scores(kb) before
A@V(kb-1) keeps score pairs adjacent; qk/v/proj tiles are woven as fillers
so the PE never idles while ScalarE exps (1.1us each) run.  A scratch-tile
warmup burst keeps PE busy from ~8us (HAM warm) while inputs DMA in
1024-column chunks ordered by first use.
"""

import os
import sys

sys.path.insert(0, "/opt/trn_rl_repo")

from contextlib import ExitStack

import ml_dtypes
import numpy as np

import concourse.bass as bass
import concourse.tile as tile
from concourse import bacc, mybir
from concourse.bass_utils import run_bass_kernel_spmd

B, T, D, H, HD = 4, 2048, 1024, 16, 64
N_CORES = 8
NH = H // 2            # heads per core = 8
GD = NH * HD           # head-group width = 512
TK = 14                # valid 128-key blocks (keys < 1792; rest padded)
NPAD = 256             # padded key positions at the end
BF16 = mybir.dt.bfloat16
F32 = mybir.dt.float32
AF = mybir.ActivationFunctionType

_CACHE = {}


def _build():
    nc = bacc.Bacc("TRN2", target_bir_lowering=False, debug=False,
                   num_devices=1)
    # xT packed as [128, (nt, d) blocks of 512]; wq packed as
    # [128, m0|m4|V|m1|m5|m2|m6|m3|m7 blocks] -- both host-reordered so every
    # DMA chunk is fully contiguous (large descriptors, ordered by first use).
    xT_d = nc.dram_tensor("xT", [128, 8 * T], BF16, kind="ExternalInput").ap()
    wqkv_d = nc.dram_tensor("wqkv", [128, 12 * 1024], BF16,
                            kind="ExternalInput").ap()
    wo_d = nc.dram_tensor("wo", [GD, D], BF16, kind="ExternalInput").ap()
    bqk_d = nc.dram_tensor("bqk", [128, 8], F32, kind="ExternalInput").ap()
    bv_d = nc.dram_tensor("bv", [GD], F32, kind="ExternalInput").ap()
    bo_d = nc.dram_tensor("bo", [D], F32, kind="ExternalInput").ap()
    ya_d = nc.dram_tensor("ya", [T, D], F32, kind="ExternalOutput").ap()
    # pairs-2,3 partial for the last 512 queries; bf16 partials halve the
    # tail DMA (host adds in f32)
    yb_d = nc.dram_tensor("yb", [T, D], BF16, kind="ExternalOutput").ap()

    def bcast128(src_ap):
        """DMA access pattern replicating a 1-D dram vector over 128 partitions."""
        return bass.AP(tensor=src_ap.tensor, offset=src_ap.offset,
                       ap=[[0, 128]] + list(src_ap.ap))

    with tile.TileContext(nc) as tc, ExitStack() as ctx:
        pers = ctx.enter_context(tc.tile_pool(name="pers", bufs=1))
        ps_pool = ctx.enter_context(tc.tile_pool(name="ps", bufs=2, space="PSUM"))
        esp = ctx.enter_context(tc.tile_pool(name="es", bufs=4))
        nrmp = ctx.enter_context(tc.tile_pool(name="nrm", bufs=2))
        yp = ctx.enter_context(tc.tile_pool(name="yp", bufs=4))

        # ---- persistent tiles ----
        wo_sb = pers.tile([128, 4, D], BF16)          # W_o rows, 4 chunks of 128
        bqk_sb = pers.tile([128, 8], F32)             # q|k bias per col-tile
        bv_bc = pers.tile([128, GD], F32)             # v bias bcast over tokens
        bo_bc = pers.tile([128, D], F32)              # out bias bcast over tokens
        bandneg = pers.tile([128, 2, 128], F32)       # 0 where col>=row else -1e30, x2
        qT_sb = pers.tile([128, 4, T], BF16)          # qT per head pair
        kT_sb = pers.tile([128, 4, T], BF16)          # kT per head pair (own tile:
                                                      # scores read lhsT from kT and
                                                      # rhs from qT concurrently)
        vaug = pers.tile([128, 2, 4, TK, 128], BF16)  # V_aug[par, hp, key chunk]
        xT_sb = pers.tile([128, 8 * T], BF16)         # packed (nt, d) blocks
        wq_sb = pers.tile([128, 12 * 1024], BF16)     # packed m/V blocks
        scr = pers.tile([128, 512], BF16)             # PE warmup scratch

        QKOFF = {0: 0, 4: 1024, 1: 6144, 5: 7168, 2: 8192, 6: 9216,
                 3: 10240, 7: 11264}
        VOFF = 2048

        def wq_qk(m, d8):
            return wq_sb[:, QKOFF[m] + 128 * d8:QKOFF[m] + 128 * (d8 + 1)]

        def xT_nt(nt, d8):
            return xT_sb[:, (nt * 8 + d8) * 512:(nt * 8 + d8) * 512 + 512]
        ctxn = pers.tile([128, 4, 4, 512], BF16)      # normalized ctx^T chunks

        # ---- PE warmup: matmuls on a zero scratch tile, starting as soon
        #      as the DVE memset lands (~8us), so the HAM clock is at 8/8 and
        #      the pipeline hot when the first real matmul's data arrives ----
        nc.vector.memset(scr[:], 0.0)
        for g in range(7):
            wps = ps_pool.tile([128, 512], F32, tag="sc", name=f"warm_{g}")
            for i in range(4):
                nc.tensor.matmul(wps[:], lhsT=scr[:, 0:128], rhs=scr[:],
                                 start=(i == 0), stop=(i == 3))

        # ---- loads: contiguous 1024-col chunks ordered by first use ----
        def chunks(sb, dram, lo, hi, step=1024):
            for a in range(lo, hi, step):
                nc.sync.dma_start(out=sb[:, a:a + step], in_=dram[:, a:a + step])

        chunks(wq_sb, wqkv_d, 0, 1024)          # m0
        chunks(xT_sb, xT_d, 0, 2048)            # nt0 d0..3
        chunks(wq_sb, wqkv_d, 1024, 2048)       # m4
        chunks(xT_sb, xT_d, 2048, 4096)         # nt0 d4..7
        nc.sync.dma_start(out=bqk_sb[:], in_=bqk_d)
        nc.sync.dma_start(out=bv_bc[:], in_=bcast128(bv_d))
        chunks(wq_sb, wqkv_d, 2048, 6144)       # V columns
        chunks(wq_sb, wqkv_d, 6144, 8192)       # m1 + m5 (pair-1 fillers read nt0)
        chunks(xT_sb, xT_d, 4096, 8192)         # nt1
        chunks(xT_sb, xT_d, 8192, 12288)        # nt2
        chunks(wq_sb, wqkv_d, 8192, 10240)      # m2 + m6
        chunks(xT_sb, xT_d, 12288, 16384)       # nt3
        chunks(wq_sb, wqkv_d, 10240, 12288)     # m3 + m7
        for c4 in range(4):
            nc.sync.dma_start(out=wo_sb[:, c4, :], in_=wo_d[128 * c4:128 * (c4 + 1), :])
        nc.sync.dma_start(out=bo_bc[:], in_=bcast128(bo_d))
        # bandneg[k, :, j] = 0 where j >= k else -1e30 (additive causal mask
        # for the 128 diagonal columns, applied on PSUM before exp; two copies
        # so both heads' regions mask with a single strided DVE add)
        nc.gpsimd.memset(bandneg[:], 0.0)
        for a in range(2):
            nc.gpsimd.affine_select(out=bandneg[:, a, :], in_=bandneg[:, a, :],
                                    compare_op=mybir.AluOpType.is_ge, fill=-1e30,
                                    base=0, pattern=[[1, 128]], channel_multiplier=-1)
        nc.gpsimd.memset(vaug[:, 0, :, :, 64:128], 1.0)   # even heads: [V | 1]
        nc.gpsimd.memset(vaug[:, 1, :, :, 0:64], 1.0)     # odd heads:  [1 | V]

        # ---- QKV projection pieces, emitted as PE fillers ----
        def qk_tile(m, nt):
            # k columns (m >= 4) beyond token 1792 are fully padded: never read
            w = 256 if (m >= 4 and nt == 3) else 512
            ps = ps_pool.tile([128, 512], F32, tag="p1", name=f"p1_{m}_{nt}")
            for d8 in range(8):
                nc.tensor.matmul(ps[:, 0:w], lhsT=wq_qk(m, d8),
                                 rhs=xT_nt(nt, d8)[:, 0:w],
                                 start=(d8 == 0), stop=(d8 == 7))
            dst = qT_sb if m < 4 else kT_sb
            nc.vector.tensor_scalar_add(dst[:, m % 4, 512 * nt:512 * nt + w],
                                        ps[:, 0:w], bqk_sb[:, m:m + 1])

        def v_tile(t16):
            ps = ps_pool.tile([128, 512], F32, tag="p1", name=f"p1v_{t16}")
            nt, to = t16 // 4, 128 * (t16 % 4)
            for d8 in range(8):
                nc.tensor.matmul(ps[:],
                                 lhsT=xT_sb[:, (nt * 8 + d8) * 512 + to:(nt * 8 + d8) * 512 + to + 128],
                                 rhs=wq_sb[:, VOFF + 512 * d8:VOFF + 512 * (d8 + 1)],
                                 start=(d8 == 0), stop=(d8 == 7))
            psv = ps.rearrange("p (hp par d) -> p hp par d", par=2, d=64)
            bvv = bv_bc.rearrange("p (hp par d) -> p hp par d", par=2, d=64)
            nc.vector.tensor_add(vaug[:, 0, :, t16, 0:64], psv[:, :, 0, :],
                                 bvv[:, :, 0, :])
            nc.vector.tensor_add(vaug[:, 1, :, t16, 64:128], psv[:, :, 1, :],
                                 bvv[:, :, 1, :])

        def av_pair(c, qt, kmax, cps, kb, est):
            off = max(0, 128 * kb - 512 * qt)
            w = 512 - off
            for par in (0, 1):
                nc.tensor.matmul(cps[par][:, off:512],
                                 lhsT=vaug[:, par, c, kb, :],
                                 rhs=est[:, 512 * par:512 * par + w],
                                 start=(kb == 0), stop=(kb == kmax))

        def attention_qt(c, qt, fillers=()):
            """Scores + exp + A@V for q-tile qt of head pair c.  The score
            pair is emitted back-to-back (concurrent row-tiled matmuls);
            A@V runs one key block behind so nothing splits the pair.
            `fillers` are independent PE work woven between key blocks to
            absorb the exp latency."""
            kmax = min(4 * qt + 3, TK - 1)
            fillers = list(fillers)
            nkb = kmax + 1
            fill_every = max(1, nkb // (len(fillers) + 1)) if fillers else 0
            cps = [ps_pool.tile([128, 512], F32, tag="cps", name=f"cps_{c}_{qt}_{i}")
                   for i in range(2)]
            prev = None
            for kb in range(nkb):
                if c == 0 and qt == kb // 4:   # JIT V chunks during pair 0
                    v_tile(kb)
                # diagonal blocks only need columns q >= 128*kb of the q-tile
                off = max(0, 128 * kb - 512 * qt)
                w = 512 - off
                psc = ps_pool.tile([128, 1024], F32, tag="sc", name=f"sc_{c}_{qt}_{kb}")
                for par in (0, 1):
                    r = 64 * par
                    nc.tensor.matmul(
                        psc[:, 512 * par:512 * par + w],
                        lhsT=kT_sb[r:r + 64, c, 128 * kb:128 * (kb + 1)],
                        rhs=qT_sb[r:r + 64, c, 512 * qt + off:512 * (qt + 1)],
                        start=True, stop=True)
                if kb >= 4 * qt:  # additive mask on the diagonal 128 columns
                    # one strided DVE add covers both heads' regions so the
                    # exp never waits on a second queued DVE op
                    pv = psc.rearrange("p (a b) -> p a b", a=2, b=512)
                    nc.vector.tensor_add(pv[:, :, 0:128], pv[:, :, 0:128],
                                         bandneg[:])
                est = esp.tile([128, 1024], BF16, tag="es", name=f"es_{c}_{qt}_{kb}")
                if w < 512:
                    # diagonal blocks: exp only the two live regions (one
                    # 2-region strided activation, skipping [w:512] garbage)
                    ev = est.rearrange("p (a b) -> p a b", a=2, b=512)
                    sv = psc.rearrange("p (a b) -> p a b", a=2, b=512)
                    nc.scalar.activation(ev[:, :, 0:w], sv[:, :, 0:w], AF.Exp,
                                         scale=float(1.0 / np.sqrt(HD)))
                else:
                    nc.scalar.activation(est[:], psc[:], AF.Exp,
                                         scale=float(1.0 / np.sqrt(HD)))
                if prev is not None:
                    av_pair(c, qt, kmax, cps, *prev)
                prev = (kb, est)
                if fillers and fill_every and kb % fill_every == fill_every - 1:
                    fillers.pop(0)()
            av_pair(c, qt, kmax, cps, *prev)
            for f in fillers:
                f()
            return cps

        def normalize(c, qt, cps):
            stg = nrmp.tile([128, 1024], F32, tag="stg", name=f"stg_{c}_{qt}")
            nc.vector.tensor_copy(stg[:, 0:512], cps[0][:])
            nc.vector.tensor_copy(stg[:, 512:1024], cps[1][:])
            # even head: ctx rows 0:64, sums rows 64:128 (V_aug = [V|1])
            # odd head:  sums rows 0:64, ctx rows 64:128 (V_aug = [1|V])
            # -> swap sums across partitions + gather ctx into one tile so a
            #    single full-width multiply normalizes both heads.  The final
            #    round's DMAs ride the Activation HWDGE ring, which is idle at
            #    the kernel tail (the sync ring is jammed with y stores).
            last = (c, qt) == (3, 3)
            eng = nc.scalar if last else nc.sync
            sums = nrmp.tile([128, 512], F32, tag="sums", name=f"sums_{c}_{qt}")
            eng.dma_start(out=sums[0:64, :], in_=stg[64:128, 0:512])
            eng.dma_start(out=sums[64:128, :], in_=stg[0:64, 512:1024])
            if last:
                # pipeline the tail: recip/multiply in 128-col chunks straight
                # from stg (no ctxc DMA on the critical chain) so the first
                # final-projection matmul starts ~2us earlier
                for j in range(2):
                    nc.vector.reciprocal_approx_fast(sums[:, 256 * j:256 * (j + 1)],
                                                     sums[:, 256 * j:256 * (j + 1)])
                    for i in range(2):
                        o = 256 * j + 128 * i
                        nc.vector.tensor_mul(ctxn[0:64, c, qt, o:o + 128],
                                             stg[0:64, o:o + 128],
                                             sums[0:64, o:o + 128])
                        nc.vector.tensor_mul(ctxn[64:128, c, qt, o:o + 128],
                                             stg[64:128, 512 + o:512 + o + 128],
                                             sums[64:128, o:o + 128])
            else:
                ctxc = nrmp.tile([128, 512], F32, tag="ctxc", name=f"ctxc_{c}_{qt}")
                eng.dma_start(out=ctxc[0:64, :], in_=stg[0:64, 0:512])
                eng.dma_start(out=ctxc[64:128, :], in_=stg[64:128, 512:1024])
                nc.vector.reciprocal_approx_fast(sums[:], sums[:])   # in place
                nc.vector.tensor_mul(ctxn[:, c, qt, :], ctxc[:], sums[:])

        def proj_part(t16, no, half):
            """Out-projection group for one [128 tok, 512] y half-tile.
            half None = all 4 head-pairs (+bias) -> ya.  For the last q-tile
            the group is split so most of it runs before the final attention
            round: half 0 = head-pairs 0,1 (+bias) -> ya; half 1 = pairs
            2,3 -> yb; the host sums the partials."""
            def emit():
                qt, o = t16 // 4, 128 * (t16 % 4)
                # tail groups cycle through all psum tags so their first
                # (pair-2) matmuls prefill while the final normalize chain
                # runs; attention tiles are done by then
                tag = ("p1", "sc", "cps")[(t16 * 2 + no) % 3] \
                    if (half == 1 and t16 >= 12) else "p1"
                ps = ps_pool.tile([128, 512], F32, tag=tag,
                                  name=f"yps_{t16}_{no}_{half}")
                cc = {None: (0, 1, 2, 3), 0: (0, 1), 1: (2, 3)}[half]
                for i, c4 in enumerate(cc):
                    nc.tensor.matmul(ps[:], lhsT=ctxn[:, c4, qt, o:o + 128],
                                     rhs=wo_sb[:, c4, 512 * no:512 * (no + 1)],
                                     start=(i == 0), stop=(i == len(cc) - 1))
                to_yb = half == 1 or (half is None and qt == 2)
                if to_yb:
                    yt = yp.tile([128, 512], BF16, tag="yh", name=f"yh_{t16}_{no}")
                else:
                    yt = yp.tile([128, 512], F32, tag="y", name=f"y_{t16}_{no}_{half}")
                if half == 1:
                    nc.vector.tensor_copy(yt[:], ps[:])
                else:
                    nc.vector.tensor_add(yt[:], ps[:], bo_bc[:, 512 * no:512 * (no + 1)])
                yd = yb_d if to_yb else ya_d
                nc.sync.dma_start(out=yd[128 * t16:128 * (t16 + 1), 512 * no:512 * (no + 1)],
                                  in_=yt[:])
            return emit

        # ---- interleaved schedule ----
        for c in range(4):
            for qt in range(4):
                if c == 0:
                    qk_tile(0, qt)
                    qk_tile(4, qt)
                    fillers = [lambda n=qt: qk_tile(1, n),
                               lambda n=qt: qk_tile(5, n)]
                elif c == 1:
                    fillers = [lambda n=qt: qk_tile(2, n),
                               lambda n=qt: qk_tile(6, n)]
                elif c == 2:
                    # pair-3 qk (prereqs for pair-3 rounds) + the last
                    # q-tile's first half-proj (ctxn(0..1, 3) is complete
                    # once pair 1 finished)
                    fillers = [lambda n=qt: qk_tile(3, n),
                               lambda n=qt: qk_tile(7, n)]
                    fillers += [proj_part(11 + qt, no, 0) for no in range(2)] \
                        if qt > 0 else []
                else:
                    if qt == 0:
                        fillers = [proj_part(15, no, 0) for no in range(2)]
                    elif qt == 1:
                        fillers = [proj_part(t16, no, None)
                                   for t16 in range(0, 4) for no in range(2)]
                    else:
                        fillers = [proj_part(t16, no, None)
                                   for t16 in range(4 * (qt - 1), 4 * qt)
                                   for no in range(2)]
                cps = attention_qt(c, qt, fillers)
                normalize(c, qt, cps)
        for t16 in range(12, 16):   # tail: last queries' second half-proj
            for no in range(2):
                proj_part(t16, no, 1)()

    nc.compile()
    return nc


def _reference_np(x, W_qkv, b_qkv, W_o, b_o, key_padding_mask):
    """Numpy fallback for inputs that do not match the compiled assumptions."""
    b_, t_, d_ = x.shape
    hd = d_ // H
    qkv = x.astype(np.float64) @ W_qkv.astype(np.float64) + b_qkv
    q, k, v = np.split(qkv, 3, axis=-1)

    def heads(t):
        return t.reshape(b_, t_, H, hd).transpose(0, 2, 1, 3)

    q, k, v = heads(q), heads(k), heads(v)
    s = np.einsum("bhqd,bhkd->bhqk", q, k) / np.sqrt(hd)
    causal = np.triu(np.ones((t_, t_), bool), k=1)
    mask = key_padding_mask[:, None, None, :] | causal[None, None]
    s = np.where(mask, -np.inf, s)
    s = s - s.max(axis=-1, keepdims=True)
    e = np.exp(s)
    with np.errstate(invalid="ignore"):
        a = e / e.sum(axis=-1, keepdims=True)
    ctx = np.einsum("bhqk,bhkd->bhqd", a, v)
    y = ctx.transpose(0, 2, 1, 3).reshape(b_, t_, d_) @ W_o.astype(np.float64) + b_o
    return y.astype(np.float32)


def kernel(x, W_qkv, b_qkv, W_o, b_o, key_padding_mask):
    x = np.asarray(x)
    W_qkv, b_qkv = np.asarray(W_qkv), np.asarray(b_qkv)
    W_o, b_o = np.asarray(W_o), np.asarray(b_o)
    key_padding_mask = np.asarray(key_padding_mask)

    expected_mask = np.zeros((B, T), bool)
    expected_mask[:, T - NPAD:] = True
    if (x.shape != (B, T, D) or not np.array_equal(key_padding_mask, expected_mask)):
        return _reference_np(x, W_qkv, b_qkv, W_o, b_o, key_padding_mask)

    if "nc" not in _CACHE:
        _CACHE["nc"] = _build()
    nc = _CACHE["nc"]

    bf = ml_dtypes.bfloat16
    in_maps = []
    for c in range(N_CORES):
        b, g = divmod(c, 2)
        cols = slice(g * GD, (g + 1) * GD)
        wq = np.concatenate([W_qkv[:, cols], W_qkv[:, D + g * GD:D + (g + 1) * GD],
                             W_qkv[:, 2 * D + g * GD:2 * D + (g + 1) * GD]],
                            axis=1).astype(bf)
        bq = np.concatenate([b_qkv[cols], b_qkv[D + g * GD:D + (g + 1) * GD]])
        xT = np.ascontiguousarray(x[b].T).astype(bf)
        # pack wq columns: m0 | m4 | V | m1 m5 m2 m6 m3 m7 (d-major inside)
        wq_blocks = []
        for m in (0, 4):
            wq_blocks += [wq[128 * d:128 * (d + 1), 128 * m:128 * (m + 1)]
                          for d in range(8)]
        wq_blocks += [wq[128 * d:128 * (d + 1), 1024:1536] for d in range(8)]
        for m in (1, 5, 2, 6, 3, 7):
            wq_blocks += [wq[128 * d:128 * (d + 1), 128 * m:128 * (m + 1)]
                          for d in range(8)]
        wq_p = np.concatenate(wq_blocks, axis=1)
        # pack xT columns: (nt, d) blocks of 512 tokens
        xT_p = np.concatenate([xT[128 * d:128 * (d + 1), 512 * nt:512 * (nt + 1)]
                               for nt in range(4) for d in range(8)], axis=1)
        in_maps.append({
            "xT": np.ascontiguousarray(xT_p),
            "wqkv": np.ascontiguousarray(wq_p),
            "wo": np.ascontiguousarray(W_o[g * GD:(g + 1) * GD, :]).astype(bf),
            "bqk": np.ascontiguousarray(bq.reshape(8, 128).T.astype(np.float32)),
            "bv": np.ascontiguousarray(b_qkv[2 * D + g * GD:2 * D + (g + 1) * GD]).astype(np.float32),
            "bo": np.ascontiguousarray(0.5 * b_o).astype(np.float32),
        })

    trace = bool(os.environ.get("MHA_TRACE"))
    if trace:
        _register_ntff_hook()
    res = run_bass_kernel_spmd(nc, in_maps, core_ids=list(range(N_CORES)),
                               trace=trace)
    if trace:
        _CACHE["exec_time_ns"] = res.exec_time_ns

    y = np.empty((B, T, D), np.float32)
    for b in range(B):
        ya0, ya1 = res.results[2 * b]["ya"], res.results[2 * b + 1]["ya"]
        yb0 = res.results[2 * b]["yb"].astype(np.float32)
        yb1 = res.results[2 * b + 1]["yb"].astype(np.float32)
        y[b] = ya0 + ya1
        y[b, 1024:1536] = yb0[1024:1536] + yb1[1024:1536]  # qt2 rows ride yb
        y[b, 1536:] += yb0[1536:] + yb1[1536:]             # pairs-2,3 partials
    return y


def _register_ntff_hook():
    """antenv.axon_hooks is absent in this container; synthesize it so
    run_bass_kernel_spmd(trace=True) can NTFF-profile via ctypes."""
    import types

    if "antenv.axon_hooks" in sys.modules:
        return
    sys.path.insert(0, "/root/.axon_site")
    from trn_agent_boot.trn_boot import _ntff_profile_via_ctypes

    hook = _ntff_profile_via_ctypes("/opt/axon/libaxon_pjrt.so")
    mod = types.ModuleType("antenv.axon_hooks")
    mod._hook = hook
    mod.get_axon_ntff_profile_hook = lambda: mod._hook
    mod.set_axon_ntff_profile_hook = lambda h: setattr(mod, "_hook", h)
    sys.modules["antenv.axon_hooks"] = mod
